# revision 71
# baseline (speedup 1.0000x reference)
"""Trainium2 Bass kernel for a single-head transformer decoder layer.

Model (per batch element, S=2048, E=1024, F=4096):
    xn  = LN(tgt);  sa = causal_attn(xn)       ; h1  = tgt + sa
    xn2 = LN(h1);   ca = cross_attn(xn2, src)  ; h2  = h1 + ca
    xn3 = LN(h2);   ff = relu(xn3@W1.T)@W2.T   ; out = h2 + ff

Sharding: 8 cores = 4 batches x 2-way query-row split.  Core c owns batch
b=c//2 and, within each 512-row chunk rc, the two interleaved 128-row
groups {4rc+h, 4rc+2+h} (h=c%2, zig-zag so causal work is balanced).
The host PERMUTES each 512-row chunk so the core's own 256 rows come
first — the program is identical on every core; only DMA'd data differs.
That makes Q projection read the already-LN'd xn chunk tiles directly
(owned cols = first 256 of each chunk), eliminating the v1 kernel's
separate owned-row LN pass.  K/V over all 2048 rows are duplicated within
each pair; no collectives.

On-chip layout: activations stored transposed [feature(part), row(free)];
every matmul runs without on-chip transposes.  LN gains are folded into
weights on the host; biases are all zero.

fp8 (TRN e4m3) with DoubleRow perf mode on all six Q/K/V projections:
LN outputs (xn/xn2) and src are quantized to fp8; q/k/v weights are
host-scaled x32 into e4m3's normal range.  The x32 scales and the
1/sqrt(E) score scale are undone for free — q,k stay x32 so scores come
out x32768, folded into the softmax exp's scale; v stays x32, folded
into the attention-output PSUM copy (x1/32).  Scores/AV/O-proj/FFN stay
bf16: measured on HW, going fp8 there too reaches 1.7e-2 rel err (vs
the 2e-2 gate) for only ~15% more speed — not worth the risk; this
config measures 9.6e-3 (2x margin).

Scheduling: the Tile scheduler is a per-engine priority-heap list
scheduler (priority = emission order), so critical-path work is emitted
first and independent filler GEMMs (K2/V2 from src) are interleaved
into the O1/LN2/Q2 chain emissions to fill PE bubbles and keep the HAM
clock warm.  Weight blocks are streamed (block-major DRAM layouts).
All ACT functions are pinned to one LUT table set (see below) — the
default chooser reloads tables (1.28us) between ln/exp ops otherwise.
rsqrt and 1/x are computed as exp(-0.5*ln(x)) / exp(-ln(x)) on ACT:
ln+exp live in the same table set and avoid the slow DVE-iterative
reciprocal (3.4us) on the LN/softmax chains.

Numerics: PSUM fp32, LN stats + softmax sums fp32 (A/B stat tiles
bf16), residual stream bf16 (adds in fp32), output fp32.
Measured: ~850us/core (worst ~880k ns), rel err 9.627e-03.
"""

import os
import sys

import numpy as np

for _p in ("/opt/trn_rl_repo", "/root/.axon_site/_ro/trn_rl_repo"):
    if os.path.isdir(_p) and _p not in sys.path:
        sys.path.insert(0, _p)

import ml_dtypes  # noqa: E402

import concourse.bass as bass  # noqa: E402
import concourse.tile as tile  # noqa: E402
from concourse import bacc, mybir  # noqa: E402
from concourse.bass_utils import run_bass_kernel_spmd  # noqa: E402

# Pin every ACT function to the one table set that holds all of ours
# (exp, ln, square, copy, relu, identity).  The default per-instruction
# chooser assigns exp->exp_and_others but ln->natural_log, which makes the
# ACT engine reload its LUT (1.28us) between almost every pair of ops in
# LN/softmax chains.  Emptying the other sets (indices preserved, so the
# emitted act_func_set_id still matches the canonical act_info.json)
# forces one table for the whole kernel: zero reloads after startup.
import concourse.hw_specs as _hw_specs  # noqa: E402
import concourse.bass_interp as _interp  # noqa: E402

_ORIG_GAT = _hw_specs.get_activation_tables
_KEEP_SET = "natural_log_exp_and_others"


def _pinned_activation_tables(arch):
    tabs = _ORIG_GAT(arch)
    return {k: (v if k == _KEEP_SET else set()) for k, v in tabs.items()}


bacc.get_activation_tables = _pinned_activation_tables
_interp.get_activation_tables = _pinned_activation_tables

E = 1024
S = 2048
B = 4
F = 4096
P = 128
NE = E // P          # 8 feature blocks
NF = F // P          # 32 ff blocks
NKB = S // P         # 16 key-row blocks
RO = 1024            # owned query rows per core
NCORES = 8

F32 = mybir.dt.float32
BF16 = mybir.dt.bfloat16
F8 = mybir.dt.float8e4        # TRN E4M3: max +-240, matches ml_dtypes e4m3
BF = ml_dtypes.bfloat16
ALU = mybir.AluOpType
ACT_F = mybir.ActivationFunctionType
DR = mybir.MatmulPerfMode.DoubleRow

NEG = -1e30

_NC_CACHE = {}
LAST_RESULTS = None  # BassKernelResults of the most recent hardware run


def _build_program():
    """Emit the single SPMD program (identical for all 8 cores)."""
    nc = bacc.Bacc(
        "TRN2",
        target_bir_lowering=False,
        debug=False,
        enable_asserts=False,
        num_devices=NCORES,
    )

    d = {}
    # tgt, permuted per 512-chunk (own rows first), chunk-major
    d["tgt_t"] = nc.dram_tensor("tgt_t", [4, P, NE, 512], BF16,
                                kind="ExternalInput")
    # raw tgt, owned rows only, t-half major (residual stream seed)
    d["tgto"] = nc.dram_tensor("tgto", [2, P, NE * 512], F32,
                               kind="ExternalInput")
    # src, natural order, 256-col chunks for streaming (fp8, x1 scale)
    d["src_t"] = nc.dram_tensor("src_t", [8, P, NE, 256], F8,
                                kind="ExternalInput")
    d["mask"] = nc.dram_tensor("mask", [2, 8, P, 512], BF16, kind="ExternalInput")
    # full-resident moving-side qkv weights (fp8, x32 scale)
    for w in ("wv1", "wv2"):
        d[w] = nc.dram_tensor(w, [P, NE, E], F8, kind="ExternalInput")
    # block-streamed stationary-side qkv weights (fp8, x32 scale)
    for w in ("wq1", "wk1", "wq2", "wk2"):
        d[w] = nc.dram_tensor(w, [NE, P, NE, P], F8, kind="ExternalInput")
    # block-streamed o-proj weights (bf16)
    for w in ("wo1", "wo2"):
        d[w] = nc.dram_tensor(w, [NE, P, NE * P], BF16, kind="ExternalInput")
    d["w1"] = nc.dram_tensor("w1", [NF, P, NE * P], BF16, kind="ExternalInput")
    d["w2"] = nc.dram_tensor("w2", [NE, P, NF * P], BF16, kind="ExternalInput")
    d["out_t"] = nc.dram_tensor("out_t", [P, NE * RO], F32, kind="ExternalOutput")

    with tile.TileContext(nc) as tc:
        _emit(tc, {k: v.ap() for k, v in d.items()})

    nc.compile()
    return nc


def _emit(tc, d):
    nc = tc.nc

    def pool(name, bufs=1, side="left"):
        return tc.alloc_tile_pool(name=name, bufs=bufs, side=side)

    # one PSUM pool; per-tag bufs; total = 8 banks
    ps = tc.alloc_tile_pool(name="ps", bufs=1, space="PSUM")

    def ps_tile(name, tag, bufs, shape=(P, 512)):
        return ps.tile(list(shape), F32, name=name, tag=tag, bufs=bufs)

    # ---------------- persistent small pools (right) -----------------------
    consts = pool("consts", side="right")
    ones_f = consts.tile([P, P], F32, name="ones_f", tag="ones_f")
    nc.vector.memset(ones_f[:], 1.0)
    ones_b = consts.tile([P, P], BF16, name="ones_b", tag="ones_b")
    nc.vector.memset(ones_b[:], 1.0)
    ones_8 = consts.tile([P, 1], F8, name="ones_8", tag="ones_8")
    nc.vector.memset(ones_8[:], 1.0)
    eps_t = consts.tile([P, 1], F32, name="eps_t", tag="eps")
    nc.vector.memset(eps_t[:], 1e-5)

    tmp = pool("tmp", bufs=1, side="right")
    sq_pool = pool("sq", bufs=2, side="right")

    # ---------------- long-lived left pools --------------------------------
    h1bp = pool("h1bp", side="left")
    h1b = h1bp.tile([P, NE * RO], BF16, name="h1b", tag="h1b")
    attnp = pool("attnp", bufs=1, side="left")      # attn accum (2 slots)
    etp = pool("etp", bufs=2, side="left")          # exp(scores), 16 KB
    invp = pool("invp", side="left")        # softmax inv + sm_sb

    k1p = pool("k1p")
    k1 = k1p.tile([P, NE, S], BF16, name="k1", tag="k1")
    v1p = pool("v1p")
    v1 = v1p.tile([P, NKB, E], BF16, name="v1", tag="v1")
    q1p = pool("q1p")
    q1 = q1p.tile([P, NE, RO], BF16, name="q1", tag="q1")

    # ---------------- right: front streams ---------------------------------
    maskp = pool("maskp", bufs=3, side="right")
    statsp = pool("statsp", side="right")
    in_pool = pool("inp", bufs=2, side="right")   # whole-chunk tiles
    xnfp = pool("xnf", bufs=3, side="right")
    wv1p = pool("wv1p", side="right")
    wv1 = wv1p.tile([P, NE, E], F8, name="wv1_sb", tag="wv")
    wk1p = pool("wk1p", bufs=3, side="right")
    wq1p = pool("wq1p", bufs=2, side="right")

    # ---------------- helpers ----------------------------------------------
    def ln_stats(get_tile, rc, stat_pool, prefix):
        """Column stats for one 512-col chunk of transposed activations.

        get_tile(eb) -> [P, 512] bf16 AP.  Returns (A, Bv): [P,512] bf16
        broadcast along partitions: A = rsqrt(var+eps), Bv = mean * A.
        """
        sum_x = ps_tile(f"{prefix}sx", "sx", 1)
        sum_xx = ps_tile(f"{prefix}sxx", "sxx", 1)
        for eb in range(NE):
            t = get_tile(eb)
            sq = sq_pool.tile([P, 512], BF16, name=f"{prefix}sq", tag="sq")
            nc.scalar.square(sq[:], t)
            nc.tensor.matmul(sum_x[:], ones_b[:], t,
                             start=(eb == 0), stop=(eb == NE - 1))
            nc.tensor.matmul(sum_xx[:], ones_b[:], sq[:],
                             start=(eb == 0), stop=(eb == NE - 1))
        mu = tmp.tile([P, 512], F32, name=f"{prefix}mu", tag="t0")
        nc.vector.tensor_scalar_mul(mu[:], sum_x[:], 1.0 / E)
        var = tmp.tile([P, 512], F32, name=f"{prefix}var", tag="t2")
        nc.vector.tensor_mul(var[:], mu[:], mu[:])
        nc.vector.scalar_tensor_tensor(
            var[:], sum_xx[:], 1.0 / E, var[:], ALU.mult, ALU.subtract)
        # rsqrt(var+eps) = exp(-0.5*ln(var+eps)) — ln and exp live in the
        # same ACT table set, so the kernel never reloads tables and no
        # (slow, DVE-iterative) reciprocal is needed
        lnv = tmp.tile([P, 512], F32, name=f"{prefix}lnv", tag="t3")
        nc.scalar.activation(lnv[:], var[:], ACT_F.Ln, bias=eps_t[:])
        a = stat_pool.tile([P, 512], BF16, name=f"{prefix}A{rc}",
                           tag=f"A{rc % 2}")
        nc.scalar.activation(a[:], lnv[:], ACT_F.Exp, scale=-0.5)
        bv = stat_pool.tile([P, 512], BF16, name=f"{prefix}B{rc}",
                            tag=f"B{rc % 2}")
        with nc.allow_low_precision(reason="bf16 LN stats, error ~2^-9"):
            nc.vector.tensor_mul(bv[:], mu[:], a[:])
        return a, bv

    def ln_apply(dst, src_ap, a, bv, prefix):
        """dst (bf16) = src*A - Bv (all-bf16 SBUF ops -> fast DVE mode)."""
        t = tmp.tile([P, 512], BF16, name=f"{prefix}ap", tag="t0b")
        with nc.allow_low_precision(reason="bf16 LN apply, error ~2^-9"):
            nc.vector.tensor_mul(t[:], src_ap, a[:])
            nc.vector.tensor_sub(dst, t[:], bv[:])

    def wblk_dma(wpool, wname, fb):
        t = wpool.tile([P, NE * P], BF16, name=f"{wname}t", tag="w")
        nc.sync.dma_start(t[:], d[wname][fb])
        return t

    def wblk8_dma(wpool, wname, fb):
        t = wpool.tile([P, NE, P], F8, name=f"{wname}t", tag="w")
        nc.sync.dma_start(t[:], d[wname][fb])
        return t

    # ---------------- front: per-chunk LN1 -> Q1/K1/V1 ---------------------
    def front_chunk(rc, post_dma=None):
        ct = in_pool.tile([P, NE, 512], BF16, name="tgt_in", tag="xin")
        nc.sync.dma_start(ct[:], d["tgt_t"][rc])
        if post_dma is not None:
            post_dma()
        a, bv = ln_stats(lambda eb: ct[:, eb, :], rc, statsp, f"l1c{rc}")
        xn = xnfp.tile([P, NE, 512], F8, name="xn_rc", tag="xn")
        for eb in range(NE):
            ln_apply(xn[:, eb, :], ct[:, eb, :], a, bv, f"l1c{rc}")
        # Q1 over owned 256 cols (front half of the permuted chunk)
        for fblk in range(NE):
            wq1t = wblk8_dma(wq1p, "wq1", fblk)
            qp = ps_tile("q1ps", "mm", 3, shape=(P, 256))
            for q in range(NE // 2):
                nc.tensor.matmul(
                    qp[:], wq1t[:, 2 * q:2 * q + 2, :],
                    xn[:, 2 * q:2 * q + 2, 0:256],
                    start=(q == 0), stop=(q == NE // 2 - 1), perf_mode=DR)
            nc.scalar.copy(q1[:, fblk, rc * 256:rc * 256 + 256], qp[:])
        # K1: all 512 cols
        for kf in range(NE):
            wk1t = wblk8_dma(wk1p, "wk1", kf)
            kp = ps_tile("kp", "mm", 3)
            for q in range(NE // 2):
                nc.tensor.matmul(
                    kp[:], wk1t[:, 2 * q:2 * q + 2, :],
                    xn[:, 2 * q:2 * q + 2, :],
                    start=(q == 0), stop=(q == NE // 2 - 1), perf_mode=DR)
            nc.scalar.copy(k1[:, kf, rc * 512:rc * 512 + 512], kp[:])
        # V1: natural layout [rows, feat]
        for rb in range(4):
            for vf in range(2):
                vp = ps_tile("vp", "mm", 3)
                for q in range(NE // 2):
                    nc.tensor.matmul(
                        vp[:],
                        xn[:, 2 * q:2 * q + 2, rb * P:rb * P + P],
                        wv1[:, 2 * q:2 * q + 2, vf * 512:vf * 512 + 512],
                        start=(q == 0), stop=(q == NE // 2 - 1), perf_mode=DR)
                nc.scalar.copy(v1[:, rc * 4 + rb, vf * 512:vf * 512 + 512],
                               vp[:])

    # ---------------- attention half ---------------------------------------
    def attn_half(q_sb, k_sb, v_sb, ext, masked, t, attn, prefix):
        """scores+softmax+AV for one 512-query half; returns inv tile.

        q/k/v/et are fp8; scores and AV run in DoubleRow mode (two
        128-row contraction blocks per matmul).
        """
        et = etp.tile([P, NKB, 512], BF16, name=f"{prefix}et", tag="et")
        for kb in range(ext):
            sp = ps_tile(f"{prefix}sp", "mm", 3)
            for eb in range(NE):
                nc.tensor.matmul(
                    sp[:],
                    k_sb[:, eb, kb * P:kb * P + P],
                    q_sb[:, eb, t * 512:t * 512 + 512],
                    start=(eb == 0), stop=(eb == NE - 1))
            if masked and kb >= 8 * t:
                mt = maskp.tile([P, 512], BF16, name=f"{prefix}mt", tag="mt")
                nc.sync.dma_start(mt[:], d["mask"][t, kb - 8 * t])
                nc.vector.tensor_add(sp[:], sp[:], mt[:])
            # q,k carry x32 weight scale and the 1/sqrt(E) fold moves here:
            # scores are x(32*32*32) -> exp scale 1/32768
            nc.scalar.activation(et[:, kb, :], sp[:], ACT_F.Exp,
                                 scale=1.0 / 32768)
        # AV first (doesn't need the denominator), softmax sum after — the
        # inv chain then overlaps the o_proj matmuls instead of gating them
        for af in range(NE):
            ap_ = ps_tile(f"{prefix}avp", "av", 2)
            for kb in range(ext):
                nc.tensor.matmul(
                    ap_[:],
                    v_sb[:, kb, af * P:af * P + P],
                    et[:, kb, :],
                    start=(kb == 0), stop=(kb == ext - 1))
            o = af * RO + t * 512
            # v carries the x32 weight scale -> undo it here
            nc.scalar.mul(attn[:, o:o + 512], ap_[:], 1.0 / 32)
        # softmax denominator via ones-matmul column sums (sm and its
        # broadcast share one PSUM bank — they are strictly sequential)
        sm = ps_tile(f"{prefix}sm", "sm", 1, shape=(1, 512))
        for kb in range(ext):
            nc.tensor.matmul(sm[:], ones_b[:, 0:1], et[:, kb, :],
                             start=(kb == 0), stop=(kb == ext - 1))
        sm_sb = invp.tile([1, 512], F32, name=f"{prefix}smsb", tag="smsb")
        nc.scalar.copy(sm_sb[:], sm[:])
        sb_ps = ps_tile(f"{prefix}smb", "sm", 1)
        nc.tensor.matmul(sb_ps[:], ones_f[0:1, :], sm_sb[:],
                         start=True, stop=True)
        # 1/x = exp(-ln(x)) — same ACT table set as everything else
        lns = tmp.tile([P, 512], F32, name=f"{prefix}lns", tag="t3")
        nc.scalar.activation(lns[:], sb_ps[:], ACT_F.Ln)
        inv = invp.tile([P, 512], BF16, name=f"{prefix}inv{t}",
                        tag=f"inv{t}")
        nc.scalar.activation(inv[:], lns[:], ACT_F.Exp, scale=-1.0)
        return inv

    def o_proj_half(attn, inv, wpool, wname, res_fn, dst, t, tag):
        """dst[:, t-half] (bf16) = (W_o.T @ attn) * inv + residual."""
        for of in range(NE):
            wt = wblk_dma(wpool, wname, of)
            op = ps_tile(f"{tag}op", "mm", 3)
            for ab in range(NE):
                nc.tensor.matmul(
                    op[:], wt[:, ab * P:ab * P + P],
                    attn[:, ab * RO + t * 512:ab * RO + t * 512 + 512],
                    start=(ab == 0), stop=(ab == NE - 1))
            cp = tmp.tile([P, 512], BF16, name=f"{tag}cp", tag="t1b")
            nc.scalar.copy(cp[:], op[:])
            t1 = tmp.tile([P, 512], BF16, name=f"{tag}on", tag="t2b")
            with nc.allow_low_precision(reason="bf16 attn out, ~2^-9"):
                nc.vector.tensor_mul(t1[:], cp[:], inv[:])
                o = of * RO + t * 512
                nc.vector.tensor_add(dst[:, o:o + 512], t1[:], res_fn(of))

    # ======================= emission =======================================
    attn1 = attnp.tile([P, NE * RO], BF16, name="attn1", tag="attn")

    # wv1 DMA after chunk 0's tiles so the first LN stats aren't delayed
    front_chunk(0, post_dma=lambda: nc.sync.dma_start(wv1[:], d["wv1"][:]))
    front_chunk(1)
    inv_sa0 = attn_half(q1, k1, v1, 8, True, 0, attn1, "sa0")
    front_chunk(2)
    front_chunk(3)
    inv_sa1 = attn_half(q1, k1, v1, 16, True, 1, attn1, "sa1")

    # release front pools (reverse alloc order per side)
    wq1p.release()
    wk1p.release()
    wv1p.release()
    xnfp.release()
    in_pool.release()
    statsp.release()
    maskp.release()
    q1p.release()
    v1p.release()
    k1p.release()

    # mid-phase pools (right), in freed front space
    stats2p = pool("stats2", side="right")
    srcs = pool("srcs", bufs=4, side="right")
    wv2p = pool("wv2p", side="right")
    wv2 = wv2p.tile([P, NE, E], F8, name="wv2_sb", tag="wv2")
    nc.sync.dma_start(wv2[:], d["wv2"][:])
    wk2p = pool("wk2p", bufs=2, side="right")
    tgtohp = pool("tgtohp", side="right")
    wq2p = pool("wq2p", bufs=2, side="right")
    wo1p = pool("wo1p", bufs=2, side="right")
    xn2p = pool("xn2p", side="right")

    def tgto_half(t):
        th = tgtohp.tile([P, NE * 512], F32, name=f"tgto{t}", tag="tgto")
        nc.sync.dma_start(th[:], d["tgto"][t])
        return th

    # left: q2/k2/v2 destination tiles (before the o1/ln2 chain, so the
    # interleaved K2/V2 filler blocks can write them)
    q2p = pool("q2p")
    q2 = q2p.tile([P, NE, RO], BF16, name="q2", tag="q2")
    k2p = pool("k2p")
    k2 = k2p.tile([P, NE, S], BF16, name="k2", tag="k2")
    v2p = pool("v2p")
    v2 = v2p.tile([P, NKB, E], BF16, name="v2", tag="v2")

    def k2v2_block(hp):
        """K2/V2 over one 512-row src block (2x256 sub-chunks), streamed."""
        stiles = []
        for sc in range(2):
            t = srcs.tile([P, NE, 256], F8, name="src_in", tag="src")
            nc.sync.dma_start(t[:], d["src_t"][2 * hp + sc])
            stiles.append(t)
        for kf in range(NE):
            wk2t = wblk8_dma(wk2p, "wk2", kf)
            for sc in range(2):
                kp = ps_tile("kp2", "mm", 3, shape=(P, 256))
                for q in range(NE // 2):
                    nc.tensor.matmul(
                        kp[:], wk2t[:, 2 * q:2 * q + 2, :],
                        stiles[sc][:, 2 * q:2 * q + 2, :],
                        start=(q == 0), stop=(q == NE // 2 - 1), perf_mode=DR)
                o = (2 * hp + sc) * 256
                nc.scalar.copy(k2[:, kf, o:o + 256], kp[:])
        for sc in range(2):
            for rb in range(2):
                for vf in range(2):
                    vp = ps_tile("vp2", "mm", 3)
                    for q in range(NE // 2):
                        nc.tensor.matmul(
                            vp[:],
                            stiles[sc][:, 2 * q:2 * q + 2, rb * P:rb * P + P],
                            wv2[:, 2 * q:2 * q + 2, vf * 512:vf * 512 + 512],
                            start=(q == 0), stop=(q == NE // 2 - 1),
                            perf_mode=DR)
                    nc.scalar.copy(
                        v2[:, (2 * hp + sc) * 2 + rb, vf * 512:vf * 512 + 512],
                        vp[:])

    def ln2_q2_half(t):
        a, bv = ln_stats(
            lambda eb: h1b[:, eb * RO + t * 512:eb * RO + t * 512 + 512],
            t, stats2p, f"l2t{t}")
        xn2 = xn2p.tile([P, NE, 512], F8, name="xn2", tag="xn2")
        for eb in range(NE):
            ln_apply(xn2[:, eb, :],
                     h1b[:, eb * RO + t * 512:eb * RO + t * 512 + 512],
                     a, bv, f"l2t{t}")
        for fblk in range(NE):
            wq2t = wblk8_dma(wq2p, "wq2", fblk)
            qp = ps_tile("q2ps", "mm", 3)
            for q in range(NE // 2):
                nc.tensor.matmul(
                    qp[:], wq2t[:, 2 * q:2 * q + 2, :],
                    xn2[:, 2 * q:2 * q + 2, :],
                    start=(q == 0), stop=(q == NE // 2 - 1), perf_mode=DR)
            nc.scalar.copy(q2[:, fblk, t * 512:t * 512 + 512], qp[:])

    # critical path interleaved with K2/V2 filler blocks: the scheduler
    # slots the filler MMs into the o_proj/LN2/Q2 dependency-chain bubbles
    th0 = tgto_half(0)
    o_proj_half(attn1, inv_sa0, wo1p, "wo1",
                lambda of: th0[:, of * 512:of * 512 + 512], h1b, 0, "o1a")
    k2v2_block(0)
    ln2_q2_half(0)
    k2v2_block(1)
    th1 = tgto_half(1)
    o_proj_half(attn1, inv_sa1, wo1p, "wo1",
                lambda of: th1[:, of * 512:of * 512 + 512], h1b, 1, "o1b")
    k2v2_block(2)
    ln2_q2_half(1)
    k2v2_block(3)

    # release pools whose last readers are now emitted (LIFO)
    xn2p.release()
    wo1p.release()
    wq2p.release()
    tgtohp.release()
    wk2p.release()
    wv2p.release()
    srcs.release()
    stats2p.release()

    # ca-phase pools
    h2bp = pool("h2bp", side="right")
    h2b = h2bp.tile([P, NE * RO], BF16, name="h2b", tag="h2b")
    stats3p = pool("stats3", side="right")
    xn3p = pool("xn3p", bufs=2, side="right")
    wo2p = pool("wo2p", bufs=2, side="right")

    attn2 = attnp.tile([P, NE * RO], BF16, name="attn2", tag="attn")

    def ln3_half(t):
        a, bv = ln_stats(
            lambda eb: h2b[:, eb * RO + t * 512:eb * RO + t * 512 + 512],
            t, stats3p, f"l3t{t}")
        xn3 = xn3p.tile([P, NE * 512], BF16, name="xn3", tag="xn3")
        for eb in range(NE):
            ln_apply(xn3[:, eb * 512:eb * 512 + 512],
                     h2b[:, eb * RO + t * 512:eb * RO + t * 512 + 512],
                     a, bv, f"l3t{t}")
        return xn3

    inv_ca0 = attn_half(q2, k2, v2, 16, False, 0, attn2, "ca0")
    o_proj_half(attn2, inv_ca0, wo2p, "wo2",
                lambda of: h1b[:, of * RO:of * RO + 512], h2b, 0, "o2a")
    xn3_0 = ln3_half(0)
    inv_ca1 = attn_half(q2, k2, v2, 16, False, 1, attn2, "ca1")
    o_proj_half(attn2, inv_ca1, wo2p, "wo2",
                lambda of: h1b[:, of * RO + 512:of * RO + 1024], h2b, 1,
                "o2b")
    xn3_1 = ln3_half(1)

    # attention no longer needed; free left space for FF hidden tiles
    wo2p.release()
    v2p.release()
    k2p.release()
    q2p.release()
    invp.release()
    etp.release()
    attnp.release()

    hft0p = pool("hft0p")
    hft0 = hft0p.tile([P, NF * 512], BF16, name="hft0", tag="hft0")
    hft1p = pool("hft1p")
    hft1 = hft1p.tile([P, NF * 512], BF16, name="hft1", tag="hft1")
    w1p = pool("w1p", bufs=3, side="right")
    outp = pool("outp", bufs=4, side="right")
    w2p = pool("w2p", bufs=2, side="right")

    def ff1_half(xn3, hft, wpool):
        for fb in range(NF):
            w1t = wpool.tile([P, NE * P], BF16, name="w1t", tag="w1")
            nc.sync.dma_start(w1t[:], d["w1"][fb])
            hps = ps_tile("hps", "mm", 3)
            for eb in range(NE):
                nc.tensor.matmul(
                    hps[:], w1t[:, eb * P:eb * P + P],
                    xn3[:, eb * 512:eb * 512 + 512],
                    start=(eb == 0), stop=(eb == NE - 1))
            nc.scalar.activation(hft[:, fb * 512:fb * 512 + 512], hps[:],
                                 ACT_F.Relu)

    def ff2_half(hft, t, wpool):  # noqa: ANN001
        for of in range(NE):
            w2t = wpool.tile([P, NF * P], BF16, name="w2t", tag="w2")
            nc.sync.dma_start(w2t[:], d["w2"][of])
            ops = ps_tile("ops", "mm", 3)
            for fb in range(NF):
                nc.tensor.matmul(
                    ops[:], w2t[:, fb * P:fb * P + P],
                    hft[:, fb * 512:fb * 512 + 512],
                    start=(fb == 0), stop=(fb == NF - 1))
            o = of * RO + t * 512
            ot = outp.tile([P, 512], F32, name="ot", tag="ot")
            nc.vector.tensor_add(ot[:], ops[:], h2b[:, o:o + 512])
            nc.sync.dma_start(d["out_t"][:, o:o + 512], ot[:])

    ff1_half(xn3_0, hft0, w1p)
    ff2_half(hft0, 0, w2p)
    ff1_half(xn3_1, hft1, w1p)
    ff2_half(hft1, 1, w2p)

    # teardown (reverse alloc order per side)
    w2p.release()
    outp.release()
    w1p.release()
    hft1p.release()
    hft0p.release()
    h1bp.release()
    xn3p.release()
    # stats3 is under xn3p? alloc order: stats3p, xn3p, wo2p (wo2p released)
    stats3p.release()
    h2bp.release()
    sq_pool.release()
    tmp.release()
    consts.release()
    ps.release()


# ---------------------------------------------------------------------------
# host side: input swizzling, weight folding, output assembly
# ---------------------------------------------------------------------------

def _swz_w(w_t):
    """[E_in, N] (already [in, out]) -> SBUF image [P, (E_in/P)*N]."""
    e_in, n = w_t.shape
    return np.ascontiguousarray(
        w_t.reshape(e_in // P, P, n).transpose(1, 0, 2).reshape(P, -1))


def _swz_blk(w_t):
    """[E_in, N] -> block-streamed [N/P, P, (E_in/P)*P] (fout-block major)."""
    e_in, n = w_t.shape
    return np.ascontiguousarray(
        w_t.reshape(e_in // P, P, n // P, P).transpose(2, 1, 0, 3)
        .reshape(n // P, P, (e_in // P) * P))


def _chunk_groups(rc, h):
    """group order within 512-row chunk rc for core-half h (own first)."""
    return [4 * rc + h, 4 * rc + 2 + h, 4 * rc + 1 - h, 4 * rc + 3 - h]


def _own_rows(h):
    """owned rows in q/attn/output column order (rc-major, 2 groups each)."""
    idx = []
    for rc in range(4):
        for g in (4 * rc + h, 4 * rc + 2 + h):
            idx.extend(range(g * P, (g + 1) * P))
    return np.array(idx)


def _key_rows(h):
    """key rows in k1/v1 column order (permuted chunks)."""
    idx = []
    for rc in range(4):
        for g in _chunk_groups(rc, h):
            idx.extend(range(g * P, (g + 1) * P))
    return np.array(idx)


def make_in_maps(inputs):
    f32 = np.float32
    tgt = np.asarray(inputs["tgt_embs"], f32)
    src = np.asarray(inputs["src_encs"], f32)

    g1 = np.asarray(inputs["ln1_g"], f32)
    g2 = np.asarray(inputs["ln2_g"], f32)
    g3 = np.asarray(inputs["ln3_g"], f32)
    for nm in ("sa_bq", "sa_bk", "sa_bv", "sa_bo", "ca_bq", "ca_bk", "ca_bv",
               "ca_bo", "ff_b1", "ff_b2", "ln1_b", "ln2_b", "ln3_b"):
        assert np.abs(np.asarray(inputs[nm])).max() == 0.0, \
            f"nonzero bias {nm} not supported"

    # qkv weights go to fp8 at x32 so they sit in e4m3's normal range; the
    # 1/sqrt(E) score scale and the x32 factors are undone on-chip (exp
    # scale=1/32768 for q*k, x1/32 on the attention-output copies)
    F8H = ml_dtypes.float8_e4m3

    def to8(x):
        return np.clip(x * 32.0, -240.0, 240.0).astype(F8H)

    wq1 = np.asarray(inputs["sa_Wq"], f32) * g1[None, :]
    wk1 = np.asarray(inputs["sa_Wk"], f32) * g1[None, :]
    wv1 = np.asarray(inputs["sa_Wv"], f32) * g1[None, :]
    wo1 = np.asarray(inputs["sa_Wo"], f32)
    wq2 = np.asarray(inputs["ca_Wq"], f32) * g2[None, :]
    wk2 = np.asarray(inputs["ca_Wk"], f32)
    wv2 = np.asarray(inputs["ca_Wv"], f32)
    wo2 = np.asarray(inputs["ca_Wo"], f32)
    w1 = np.asarray(inputs["ff_W1"], f32) * g3[None, :]
    w2 = np.asarray(inputs["ff_W2"], f32)

    w_common = {
        "wv1": _swz_w(to8(wv1.T)).reshape(P, NE, E),
        "wv2": _swz_w(to8(wv2.T)).reshape(P, NE, E),
        "wq1": _swz_blk(to8(wq1.T)).reshape(NE, P, NE, P),
        "wk1": _swz_blk(to8(wk1.T)).reshape(NE, P, NE, P),
        "wq2": _swz_blk(to8(wq2.T)).reshape(NE, P, NE, P),
        "wk2": _swz_blk(to8(wk2.T)).reshape(NE, P, NE, P),
        "wo1": _swz_blk(wo1.T.astype(BF)),
        "wo2": _swz_blk(wo2.T.astype(BF)),
        "w1": _swz_blk(w1.T.astype(BF)),
        "w2": _swz_blk(w2.T.astype(BF)),
    }

    in_maps = []
    for c in range(NCORES):
        b, h = c // 2, c % 2
        krows = _key_rows(h)
        qrows = _own_rows(h)
        # tgt permuted chunk-major [4, P, NE, 512]
        tgt_perm = tgt[b][krows].T.astype(BF)  # [E, S] in key order
        tgt_t = np.ascontiguousarray(
            tgt_perm.reshape(NE, P, 4, 512).transpose(2, 1, 0, 3))
        tgto = np.ascontiguousarray(
            tgt[b][qrows].T.reshape(NE, P, 2, 512)
            .transpose(2, 1, 0, 3).reshape(2, P, NE * 512))
        # src natural order, 256-col chunks [8, P, NE, 256], fp8 x1
        src_t = np.ascontiguousarray(
            np.clip(src[b].T, -240, 240).astype(F8H)
            .reshape(NE, P, 8, 256).transpose(2, 1, 0, 3))
        # causal masks: key rows kr vs query rows qg (both permuted orders)
        mask = np.zeros((2, 8, P, 512), np.float32)
        for t in range(2):
            qg = qrows[t * 512:(t + 1) * 512]
            for kb in range(8):
                kr = krows[(8 * t + kb) * P:(8 * t + kb + 1) * P]
                mask[t, kb] = np.where(kr[:, None] <= qg[None, :], 0.0, NEG)
        in_maps.append({
            "tgt_t": tgt_t,
            "tgto": tgto,
            "src_t": src_t,
            "mask": mask.astype(BF),
            **w_common,
        })
    return in_maps


def assemble_output(results):
    out = np.empty((B, S, E), np.float32)
    for c in range(NCORES):
        b, h = c // 2, c % 2
        arr = np.asarray(results[c]["out_t"])  # [P, NE*RO]
        a = arr.reshape(P, NE, 8, P).transpose(2, 3, 1, 0).reshape(8, P, E)
        qrows = _own_rows(h)
        for j in range(8):
            out[b, qrows[j * P:(j + 1) * P], :] = a[j]
    return out


def get_nc():
    if "nc" not in _NC_CACHE:
        _NC_CACHE["nc"] = _build_program()
    return _NC_CACHE["nc"]


def _axon_reset():
    """Recover a wedged remote NeuronCore (NRT_EXEC_UNIT_UNRECOVERABLE)."""
    try:
        import ctypes
        lib = ctypes.CDLL("/opt/axon/libaxon_pjrt.so")
        lib.axon_reset.restype = ctypes.c_int64
        lib.axon_reset()
    except Exception:
        pass


def kernel(**inputs):
    global LAST_RESULTS
    in_maps = make_in_maps(inputs)
    nc = get_nc()
    last_err = None
    for attempt in range(3):
        try:
            res = run_bass_kernel_spmd(nc, in_maps, list(range(NCORES)))
            break
        except Exception as e:  # wedged device -> reset + retry
            last_err = e
            _axon_reset()
    else:
        raise last_err
    LAST_RESULTS = res
    return assemble_output(res.results)


# revision 72
# speedup vs baseline: 1.0504x; 1.0504x over previous
"""Trainium2 Bass kernel for a single-head transformer decoder layer.

Model (per batch element, S=2048, E=1024, F=4096):
    xn  = LN(tgt);  sa = causal_attn(xn)       ; h1  = tgt + sa
    xn2 = LN(h1);   ca = cross_attn(xn2, src)  ; h2  = h1 + ca
    xn3 = LN(h2);   ff = relu(xn3@W1.T)@W2.T   ; out = h2 + ff

Sharding: 8 cores = 4 batches x 2-way query-row split.  Core c owns batch
b=c//2 and, within each 512-row chunk rc, the two interleaved 128-row
groups {4rc+h, 4rc+2+h} (h=c%2, zig-zag so causal work is balanced).
The host PERMUTES each 512-row chunk so the core's own 256 rows come
first — the program is identical on every core; only DMA'd data differs.
That makes Q projection read the already-LN'd xn chunk tiles directly
(owned cols = first 256 of each chunk), eliminating the v1 kernel's
separate owned-row LN pass.  K/V over all 2048 rows are duplicated within
each pair; no collectives.

On-chip layout: activations stored transposed [feature(part), row(free)];
every matmul runs without on-chip transposes.  LN gains are folded into
weights on the host; biases are all zero.

fp8 (TRN e4m3) with DoubleRow perf mode on all six Q/K/V projections:
LN outputs (xn/xn2) and src are quantized to fp8; q/k/v weights are
host-scaled x32 into e4m3's normal range.  The x32 scales and the
1/sqrt(E) score scale are undone for free — q,k stay x32 so scores come
out x32768, folded into the softmax exp's scale; v stays x32, folded
into the attention-output PSUM copy (x1/32).  Scores/AV/O-proj/FFN stay
bf16: measured on HW, going fp8 there too reaches 1.7e-2 rel err (vs
the 2e-2 gate) for only ~15% more speed — not worth the risk; this
config measures 9.6e-3 (2x margin).

Scheduling: the Tile scheduler is a per-engine priority-heap list
scheduler (priority = emission order), so critical-path work is emitted
first and independent filler GEMMs (K2/V2 from src) are interleaved
into the O1/LN2/Q2 chain emissions to fill PE bubbles and keep the HAM
clock warm.  Weight blocks are streamed (block-major DRAM layouts).
All ACT functions are pinned to one LUT table set (see below) — the
default chooser reloads tables (1.28us) between ln/exp ops otherwise.
rsqrt and 1/x are computed as exp(-0.5*ln(x)) / exp(-ln(x)) on ACT:
ln+exp live in the same table set and avoid the slow DVE-iterative
reciprocal (3.4us) on the LN/softmax chains.

Numerics: PSUM fp32, LN stats + softmax sums fp32 (A/B stat tiles
bf16), residual stream bf16 (adds in fp32), output fp32.
Measured: ~850us/core (worst ~880k ns), rel err 9.627e-03.
"""

import os
import sys

import numpy as np

for _p in ("/opt/trn_rl_repo", "/root/.axon_site/_ro/trn_rl_repo"):
    if os.path.isdir(_p) and _p not in sys.path:
        sys.path.insert(0, _p)

import ml_dtypes  # noqa: E402

import concourse.bass as bass  # noqa: E402
import concourse.tile as tile  # noqa: E402
from concourse import bacc, mybir  # noqa: E402
from concourse.bass_utils import run_bass_kernel_spmd  # noqa: E402

# Pin every ACT function to the one table set that holds all of ours
# (exp, ln, square, copy, relu, identity).  The default per-instruction
# chooser assigns exp->exp_and_others but ln->natural_log, which makes the
# ACT engine reload its LUT (1.28us) between almost every pair of ops in
# LN/softmax chains.  Emptying the other sets (indices preserved, so the
# emitted act_func_set_id still matches the canonical act_info.json)
# forces one table for the whole kernel: zero reloads after startup.
import concourse.hw_specs as _hw_specs  # noqa: E402
import concourse.bass_interp as _interp  # noqa: E402

_ORIG_GAT = _hw_specs.get_activation_tables
_KEEP_SET = "natural_log_exp_and_others"


def _pinned_activation_tables(arch):
    tabs = _ORIG_GAT(arch)
    return {k: (v if k == _KEEP_SET else set()) for k, v in tabs.items()}



E = 1024
S = 2048
B = 4
F = 4096
P = 128
NE = E // P          # 8 feature blocks
NF = F // P          # 32 ff blocks
NKB = S // P         # 16 key-row blocks
RO = 1024            # owned query rows per core
NCORES = 8

F32 = mybir.dt.float32
BF16 = mybir.dt.bfloat16
F8 = mybir.dt.float8e4        # TRN E4M3: max +-240, matches ml_dtypes e4m3
BF = ml_dtypes.bfloat16
ALU = mybir.AluOpType
ACT_F = mybir.ActivationFunctionType
DR = mybir.MatmulPerfMode.DoubleRow

NEG = -1e30

_NC_CACHE = {}
LAST_RESULTS = None  # BassKernelResults of the most recent hardware run


def _build_program():
    """Emit the single SPMD program (identical for all 8 cores)."""
    bacc.get_activation_tables = _pinned_activation_tables
    _interp.get_activation_tables = _pinned_activation_tables
    try:
        return _build_program_inner()
    finally:
        bacc.get_activation_tables = _ORIG_GAT
        _interp.get_activation_tables = _ORIG_GAT


def _build_program_inner():
    nc = bacc.Bacc(
        "TRN2",
        target_bir_lowering=False,
        debug=False,
        enable_asserts=False,
        num_devices=NCORES,
    )

    d = {}
    # tgt, permuted per 512-chunk (own rows first), chunk-major
    d["tgt_t"] = nc.dram_tensor("tgt_t", [4, P, NE, 512], BF16,
                                kind="ExternalInput")
    # raw tgt, owned rows only, t-half major (residual stream seed)
    d["tgto"] = nc.dram_tensor("tgto", [2, P, NE * 512], F32,
                               kind="ExternalInput")
    # src, natural order, 256-col chunks for streaming (fp8, x1 scale)
    d["src_t"] = nc.dram_tensor("src_t", [8, P, NE, 256], F8,
                                kind="ExternalInput")
    d["mask"] = nc.dram_tensor("mask", [2, 8, P, 512], BF16, kind="ExternalInput")
    # full-resident moving-side qkv weights (fp8, x32 scale)
    for w in ("wv1", "wv2"):
        d[w] = nc.dram_tensor(w, [P, NE, E], F8, kind="ExternalInput")
    # block-streamed stationary-side qkv weights (fp8, x32 scale)
    for w in ("wq1", "wk1", "wq2", "wk2"):
        d[w] = nc.dram_tensor(w, [NE, P, NE, P], F8, kind="ExternalInput")
    # block-streamed o-proj weights (bf16)
    for w in ("wo1", "wo2"):
        d[w] = nc.dram_tensor(w, [NE, P, NE * P], BF16, kind="ExternalInput")
    d["w1"] = nc.dram_tensor("w1", [NF, P, NE * P], BF16, kind="ExternalInput")
    d["w2"] = nc.dram_tensor("w2", [NE, P, NF * P], BF16, kind="ExternalInput")
    d["out_t"] = nc.dram_tensor("out_t", [P, NE * RO], F32, kind="ExternalOutput")

    with tile.TileContext(nc) as tc:
        _emit(tc, {k: v.ap() for k, v in d.items()})

    nc.compile()
    return nc


def _emit(tc, d):
    nc = tc.nc

    def pool(name, bufs=1, side="left"):
        return tc.alloc_tile_pool(name=name, bufs=bufs, side=side)

    # one PSUM pool; per-tag bufs; total = 8 banks
    ps = tc.alloc_tile_pool(name="ps", bufs=1, space="PSUM")

    def ps_tile(name, tag, bufs, shape=(P, 512)):
        return ps.tile(list(shape), F32, name=name, tag=tag, bufs=bufs)

    # ---------------- persistent small pools (right) -----------------------
    consts = pool("consts", side="right")
    ones_f = consts.tile([P, P], F32, name="ones_f", tag="ones_f")
    nc.vector.memset(ones_f[:], 1.0)
    ones_b = consts.tile([P, P], BF16, name="ones_b", tag="ones_b")
    nc.vector.memset(ones_b[:], 1.0)
    ones_8 = consts.tile([P, 1], F8, name="ones_8", tag="ones_8")
    nc.vector.memset(ones_8[:], 1.0)
    eps_t = consts.tile([P, 1], F32, name="eps_t", tag="eps")
    nc.vector.memset(eps_t[:], 1e-5)

    tmp = pool("tmp", bufs=1, side="right")
    sq_pool = pool("sq", bufs=2, side="right")

    # ---------------- long-lived left pools --------------------------------
    h1bp = pool("h1bp", side="left")
    h1b = h1bp.tile([P, NE * RO], BF16, name="h1b", tag="h1b")
    attnp = pool("attnp", bufs=1, side="left")      # attn accum (2 slots)
    etp = pool("etp", bufs=2, side="left")          # exp(scores), 16 KB
    invp = pool("invp", side="left")        # softmax inv + sm_sb

    k1p = pool("k1p")
    k1 = k1p.tile([P, NE, S], BF16, name="k1", tag="k1")
    v1p = pool("v1p")
    v1 = v1p.tile([P, NKB, E], BF16, name="v1", tag="v1")
    q1p = pool("q1p")
    q1 = q1p.tile([P, NE, RO], BF16, name="q1", tag="q1")

    # ---------------- right: front streams ---------------------------------
    maskp = pool("maskp", bufs=3, side="right")
    statsp = pool("statsp", side="right")
    in_pool = pool("inp", bufs=2, side="right")   # whole-chunk tiles
    xnfp = pool("xnf", bufs=3, side="right")
    wv1p = pool("wv1p", side="right")
    wv1 = wv1p.tile([P, NE, E], F8, name="wv1_sb", tag="wv")
    wk1p = pool("wk1p", bufs=3, side="right")
    wq1p = pool("wq1p", bufs=2, side="right")

    # ---------------- helpers ----------------------------------------------
    def ln_stats(get_tile, rc, stat_pool, prefix):
        """Column stats for one 512-col chunk of transposed activations.

        get_tile(eb) -> [P, 512] bf16 AP.  Returns (A, Bv): [P,512] bf16
        broadcast along partitions: A = rsqrt(var+eps), Bv = mean * A.
        """
        sum_x = ps_tile(f"{prefix}sx", "sx", 1)
        sum_xx = ps_tile(f"{prefix}sxx", "sxx", 1)
        for eb in range(NE):
            t = get_tile(eb)
            sq = sq_pool.tile([P, 512], BF16, name=f"{prefix}sq", tag="sq")
            nc.scalar.square(sq[:], t)
            nc.tensor.matmul(sum_x[:], ones_b[:], t,
                             start=(eb == 0), stop=(eb == NE - 1))
            nc.tensor.matmul(sum_xx[:], ones_b[:], sq[:],
                             start=(eb == 0), stop=(eb == NE - 1))
        mu = tmp.tile([P, 512], F32, name=f"{prefix}mu", tag="t0")
        nc.vector.tensor_scalar_mul(mu[:], sum_x[:], 1.0 / E)
        var = tmp.tile([P, 512], F32, name=f"{prefix}var", tag="t2")
        nc.vector.tensor_mul(var[:], mu[:], mu[:])
        nc.vector.scalar_tensor_tensor(
            var[:], sum_xx[:], 1.0 / E, var[:], ALU.mult, ALU.subtract)
        # rsqrt(var+eps) = exp(-0.5*ln(var+eps)) — ln and exp live in the
        # same ACT table set, so the kernel never reloads tables and no
        # (slow, DVE-iterative) reciprocal is needed
        lnv = tmp.tile([P, 512], F32, name=f"{prefix}lnv", tag="t3")
        nc.scalar.activation(lnv[:], var[:], ACT_F.Ln, bias=eps_t[:])
        a = stat_pool.tile([P, 512], BF16, name=f"{prefix}A{rc}",
                           tag=f"A{rc % 2}")
        nc.scalar.activation(a[:], lnv[:], ACT_F.Exp, scale=-0.5)
        bv = stat_pool.tile([P, 512], BF16, name=f"{prefix}B{rc}",
                            tag=f"B{rc % 2}")
        with nc.allow_low_precision(reason="bf16 LN stats, error ~2^-9"):
            nc.vector.tensor_mul(bv[:], mu[:], a[:])
        return a, bv

    def ln_apply(dst, src_ap, a, bv, prefix):
        """dst (bf16) = src*A - Bv (all-bf16 SBUF ops -> fast DVE mode)."""
        t = tmp.tile([P, 512], BF16, name=f"{prefix}ap", tag="t0b")
        with nc.allow_low_precision(reason="bf16 LN apply, error ~2^-9"):
            nc.vector.tensor_mul(t[:], src_ap, a[:])
            nc.vector.tensor_sub(dst, t[:], bv[:])

    def wblk_dma(wpool, wname, fb):
        t = wpool.tile([P, NE * P], BF16, name=f"{wname}t", tag="w")
        nc.sync.dma_start(t[:], d[wname][fb])
        return t

    def wblk8_dma(wpool, wname, fb):
        t = wpool.tile([P, NE, P], F8, name=f"{wname}t", tag="w")
        nc.sync.dma_start(t[:], d[wname][fb])
        return t

    # ---------------- front: per-chunk LN1 -> Q1/K1/V1 ---------------------
    def front_chunk(rc, post_dma=None):
        ct = in_pool.tile([P, NE, 512], BF16, name="tgt_in", tag="xin")
        nc.sync.dma_start(ct[:], d["tgt_t"][rc])
        if post_dma is not None:
            post_dma()
        a, bv = ln_stats(lambda eb: ct[:, eb, :], rc, statsp, f"l1c{rc}")
        xn = xnfp.tile([P, NE, 512], F8, name="xn_rc", tag="xn")
        for eb in range(NE):
            ln_apply(xn[:, eb, :], ct[:, eb, :], a, bv, f"l1c{rc}")
        # Q1 over owned 256 cols (front half of the permuted chunk)
        for fblk in range(NE):
            wq1t = wblk8_dma(wq1p, "wq1", fblk)
            qp = ps_tile("q1ps", "mm", 3, shape=(P, 256))
            for q in range(NE // 2):
                nc.tensor.matmul(
                    qp[:], wq1t[:, 2 * q:2 * q + 2, :],
                    xn[:, 2 * q:2 * q + 2, 0:256],
                    start=(q == 0), stop=(q == NE // 2 - 1), perf_mode=DR)
            nc.scalar.copy(q1[:, fblk, rc * 256:rc * 256 + 256], qp[:])
        # K1: all 512 cols
        for kf in range(NE):
            wk1t = wblk8_dma(wk1p, "wk1", kf)
            kp = ps_tile("kp", "mm", 3)
            for q in range(NE // 2):
                nc.tensor.matmul(
                    kp[:], wk1t[:, 2 * q:2 * q + 2, :],
                    xn[:, 2 * q:2 * q + 2, :],
                    start=(q == 0), stop=(q == NE // 2 - 1), perf_mode=DR)
            nc.scalar.copy(k1[:, kf, rc * 512:rc * 512 + 512], kp[:])
        # V1: natural layout [rows, feat]
        for rb in range(4):
            for vf in range(2):
                vp = ps_tile("vp", "mm", 3)
                for q in range(NE // 2):
                    nc.tensor.matmul(
                        vp[:],
                        xn[:, 2 * q:2 * q + 2, rb * P:rb * P + P],
                        wv1[:, 2 * q:2 * q + 2, vf * 512:vf * 512 + 512],
                        start=(q == 0), stop=(q == NE // 2 - 1), perf_mode=DR)
                nc.scalar.copy(v1[:, rc * 4 + rb, vf * 512:vf * 512 + 512],
                               vp[:])

    # ---------------- attention half ---------------------------------------
    def attn_half(q_sb, k_sb, v_sb, ext, masked, t, attn, prefix):
        """scores+softmax+AV for one 512-query half; returns inv tile.

        q/k/v/et are fp8; scores and AV run in DoubleRow mode (two
        128-row contraction blocks per matmul).
        """
        et = etp.tile([P, NKB, 512], BF16, name=f"{prefix}et", tag="et")
        for kb in range(ext):
            sp = ps_tile(f"{prefix}sp", "mm", 3)
            for eb in range(NE):
                nc.tensor.matmul(
                    sp[:],
                    k_sb[:, eb, kb * P:kb * P + P],
                    q_sb[:, eb, t * 512:t * 512 + 512],
                    start=(eb == 0), stop=(eb == NE - 1))
            if masked and kb >= 8 * t:
                mt = maskp.tile([P, 512], BF16, name=f"{prefix}mt", tag="mt")
                nc.sync.dma_start(mt[:], d["mask"][t, kb - 8 * t])
                nc.vector.tensor_add(sp[:], sp[:], mt[:])
            # q,k carry x32 weight scale and the 1/sqrt(E) fold moves here:
            # scores are x(32*32*32) -> exp scale 1/32768
            nc.scalar.activation(et[:, kb, :], sp[:], ACT_F.Exp,
                                 scale=1.0 / 32768)
        # AV first (doesn't need the denominator), softmax sum after — the
        # inv chain then overlaps the o_proj matmuls instead of gating them
        for af in range(NE):
            ap_ = ps_tile(f"{prefix}avp", "av", 2)
            for kb in range(ext):
                nc.tensor.matmul(
                    ap_[:],
                    v_sb[:, kb, af * P:af * P + P],
                    et[:, kb, :],
                    start=(kb == 0), stop=(kb == ext - 1))
            o = af * RO + t * 512
            # v carries the x32 weight scale -> undo it here
            nc.scalar.mul(attn[:, o:o + 512], ap_[:], 1.0 / 32)
        # softmax denominator via ones-matmul column sums (sm and its
        # broadcast share one PSUM bank — they are strictly sequential)
        sm = ps_tile(f"{prefix}sm", "sm", 1, shape=(1, 512))
        for kb in range(ext):
            nc.tensor.matmul(sm[:], ones_b[:, 0:1], et[:, kb, :],
                             start=(kb == 0), stop=(kb == ext - 1))
        sm_sb = invp.tile([1, 512], F32, name=f"{prefix}smsb", tag="smsb")
        nc.scalar.copy(sm_sb[:], sm[:])
        sb_ps = ps_tile(f"{prefix}smb", "sm", 1)
        nc.tensor.matmul(sb_ps[:], ones_f[0:1, :], sm_sb[:],
                         start=True, stop=True)
        # 1/x = exp(-ln(x)) — same ACT table set as everything else
        lns = tmp.tile([P, 512], F32, name=f"{prefix}lns", tag="t3")
        nc.scalar.activation(lns[:], sb_ps[:], ACT_F.Ln)
        inv = invp.tile([P, 512], BF16, name=f"{prefix}inv{t}",
                        tag=f"inv{t}")
        nc.scalar.activation(inv[:], lns[:], ACT_F.Exp, scale=-1.0)
        return inv

    def o_proj_half(attn, inv, wpool, wname, res_fn, dst, t, tag):
        """dst[:, t-half] (bf16) = (W_o.T @ attn) * inv + residual."""
        for of in range(NE):
            wt = wblk_dma(wpool, wname, of)
            op = ps_tile(f"{tag}op", "mm", 3)
            for ab in range(NE):
                nc.tensor.matmul(
                    op[:], wt[:, ab * P:ab * P + P],
                    attn[:, ab * RO + t * 512:ab * RO + t * 512 + 512],
                    start=(ab == 0), stop=(ab == NE - 1))
            cp = tmp.tile([P, 512], BF16, name=f"{tag}cp", tag="t1b")
            nc.scalar.copy(cp[:], op[:])
            t1 = tmp.tile([P, 512], BF16, name=f"{tag}on", tag="t2b")
            with nc.allow_low_precision(reason="bf16 attn out, ~2^-9"):
                nc.vector.tensor_mul(t1[:], cp[:], inv[:])
                o = of * RO + t * 512
                nc.vector.tensor_add(dst[:, o:o + 512], t1[:], res_fn(of))

    # ======================= emission =======================================
    attn1 = attnp.tile([P, NE * RO], BF16, name="attn1", tag="attn")

    # wv1 DMA after chunk 0's tiles so the first LN stats aren't delayed
    front_chunk(0, post_dma=lambda: nc.sync.dma_start(wv1[:], d["wv1"][:]))
    front_chunk(1)
    inv_sa0 = attn_half(q1, k1, v1, 8, True, 0, attn1, "sa0")
    front_chunk(2)
    front_chunk(3)
    inv_sa1 = attn_half(q1, k1, v1, 16, True, 1, attn1, "sa1")

    # release front pools (reverse alloc order per side)
    wq1p.release()
    wk1p.release()
    wv1p.release()
    xnfp.release()
    in_pool.release()
    statsp.release()
    maskp.release()
    q1p.release()
    v1p.release()
    k1p.release()

    # mid-phase pools (right), in freed front space
    stats2p = pool("stats2", side="right")
    srcs = pool("srcs", bufs=4, side="right")
    wv2p = pool("wv2p", side="right")
    wv2 = wv2p.tile([P, NE, E], F8, name="wv2_sb", tag="wv2")
    nc.sync.dma_start(wv2[:], d["wv2"][:])
    wk2p = pool("wk2p", bufs=2, side="right")
    tgtohp = pool("tgtohp", side="right")
    wq2p = pool("wq2p", bufs=2, side="right")
    wo1p = pool("wo1p", bufs=2, side="right")
    xn2p = pool("xn2p", side="right")

    def tgto_half(t):
        th = tgtohp.tile([P, NE * 512], F32, name=f"tgto{t}", tag="tgto")
        nc.sync.dma_start(th[:], d["tgto"][t])
        return th

    # left: q2/k2/v2 destination tiles (before the o1/ln2 chain, so the
    # interleaved K2/V2 filler blocks can write them)
    q2p = pool("q2p")
    q2 = q2p.tile([P, NE, RO], BF16, name="q2", tag="q2")
    k2p = pool("k2p")
    k2 = k2p.tile([P, NE, S], BF16, name="k2", tag="k2")
    v2p = pool("v2p")
    v2 = v2p.tile([P, NKB, E], BF16, name="v2", tag="v2")

    def k2v2_block(hp):
        """K2/V2 over one 512-row src block (2x256 sub-chunks), streamed."""
        stiles = []
        for sc in range(2):
            t = srcs.tile([P, NE, 256], F8, name="src_in", tag="src")
            nc.sync.dma_start(t[:], d["src_t"][2 * hp + sc])
            stiles.append(t)
        for kf in range(NE):
            wk2t = wblk8_dma(wk2p, "wk2", kf)
            for sc in range(2):
                kp = ps_tile("kp2", "mm", 3, shape=(P, 256))
                for q in range(NE // 2):
                    nc.tensor.matmul(
                        kp[:], wk2t[:, 2 * q:2 * q + 2, :],
                        stiles[sc][:, 2 * q:2 * q + 2, :],
                        start=(q == 0), stop=(q == NE // 2 - 1), perf_mode=DR)
                o = (2 * hp + sc) * 256
                nc.scalar.copy(k2[:, kf, o:o + 256], kp[:])
        for sc in range(2):
            for rb in range(2):
                for vf in range(2):
                    vp = ps_tile("vp2", "mm", 3)
                    for q in range(NE // 2):
                        nc.tensor.matmul(
                            vp[:],
                            stiles[sc][:, 2 * q:2 * q + 2, rb * P:rb * P + P],
                            wv2[:, 2 * q:2 * q + 2, vf * 512:vf * 512 + 512],
                            start=(q == 0), stop=(q == NE // 2 - 1),
                            perf_mode=DR)
                    nc.scalar.copy(
                        v2[:, (2 * hp + sc) * 2 + rb, vf * 512:vf * 512 + 512],
                        vp[:])

    def ln2_q2_half(t):
        a, bv = ln_stats(
            lambda eb: h1b[:, eb * RO + t * 512:eb * RO + t * 512 + 512],
            t, stats2p, f"l2t{t}")
        xn2 = xn2p.tile([P, NE, 512], F8, name="xn2", tag="xn2")
        for eb in range(NE):
            ln_apply(xn2[:, eb, :],
                     h1b[:, eb * RO + t * 512:eb * RO + t * 512 + 512],
                     a, bv, f"l2t{t}")
        for fblk in range(NE):
            wq2t = wblk8_dma(wq2p, "wq2", fblk)
            qp = ps_tile("q2ps", "mm", 3)
            for q in range(NE // 2):
                nc.tensor.matmul(
                    qp[:], wq2t[:, 2 * q:2 * q + 2, :],
                    xn2[:, 2 * q:2 * q + 2, :],
                    start=(q == 0), stop=(q == NE // 2 - 1), perf_mode=DR)
            nc.scalar.copy(q2[:, fblk, t * 512:t * 512 + 512], qp[:])

    # critical path interleaved with K2/V2 filler blocks: the scheduler
    # slots the filler MMs into the o_proj/LN2/Q2 dependency-chain bubbles
    th0 = tgto_half(0)
    o_proj_half(attn1, inv_sa0, wo1p, "wo1",
                lambda of: th0[:, of * 512:of * 512 + 512], h1b, 0, "o1a")
    k2v2_block(0)
    ln2_q2_half(0)
    k2v2_block(1)
    th1 = tgto_half(1)
    o_proj_half(attn1, inv_sa1, wo1p, "wo1",
                lambda of: th1[:, of * 512:of * 512 + 512], h1b, 1, "o1b")
    k2v2_block(2)
    ln2_q2_half(1)
    k2v2_block(3)

    # release pools whose last readers are now emitted (LIFO)
    xn2p.release()
    wo1p.release()
    wq2p.release()
    tgtohp.release()
    wk2p.release()
    wv2p.release()
    srcs.release()
    stats2p.release()

    # ca-phase pools
    h2bp = pool("h2bp", side="right")
    h2b = h2bp.tile([P, NE * RO], BF16, name="h2b", tag="h2b")
    stats3p = pool("stats3", side="right")
    xn3p = pool("xn3p", bufs=2, side="right")
    wo2p = pool("wo2p", bufs=2, side="right")

    attn2 = attnp.tile([P, NE * RO], BF16, name="attn2", tag="attn")

    def ln3_half(t):
        a, bv = ln_stats(
            lambda eb: h2b[:, eb * RO + t * 512:eb * RO + t * 512 + 512],
            t, stats3p, f"l3t{t}")
        xn3 = xn3p.tile([P, NE * 512], BF16, name="xn3", tag="xn3")
        for eb in range(NE):
            ln_apply(xn3[:, eb * 512:eb * 512 + 512],
                     h2b[:, eb * RO + t * 512:eb * RO + t * 512 + 512],
                     a, bv, f"l3t{t}")
        return xn3

    inv_ca0 = attn_half(q2, k2, v2, 16, False, 0, attn2, "ca0")
    o_proj_half(attn2, inv_ca0, wo2p, "wo2",
                lambda of: h1b[:, of * RO:of * RO + 512], h2b, 0, "o2a")
    xn3_0 = ln3_half(0)
    inv_ca1 = attn_half(q2, k2, v2, 16, False, 1, attn2, "ca1")
    o_proj_half(attn2, inv_ca1, wo2p, "wo2",
                lambda of: h1b[:, of * RO + 512:of * RO + 1024], h2b, 1,
                "o2b")
    xn3_1 = ln3_half(1)

    # attention no longer needed; free left space for FF hidden tiles
    wo2p.release()
    v2p.release()
    k2p.release()
    q2p.release()
    invp.release()
    etp.release()
    attnp.release()

    hft0p = pool("hft0p")
    hft0 = hft0p.tile([P, NF * 512], BF16, name="hft0", tag="hft0")
    hft1p = pool("hft1p")
    hft1 = hft1p.tile([P, NF * 512], BF16, name="hft1", tag="hft1")
    w1p = pool("w1p", bufs=3, side="right")
    outp = pool("outp", bufs=4, side="right")
    w2p = pool("w2p", bufs=2, side="right")

    def ff1_half(xn3, hft, wpool):
        for fb in range(NF):
            w1t = wpool.tile([P, NE * P], BF16, name="w1t", tag="w1")
            nc.sync.dma_start(w1t[:], d["w1"][fb])
            hps = ps_tile("hps", "mm", 3)
            for eb in range(NE):
                nc.tensor.matmul(
                    hps[:], w1t[:, eb * P:eb * P + P],
                    xn3[:, eb * 512:eb * 512 + 512],
                    start=(eb == 0), stop=(eb == NE - 1))
            nc.scalar.activation(hft[:, fb * 512:fb * 512 + 512], hps[:],
                                 ACT_F.Relu)

    def ff2_half(hft, t, wpool):  # noqa: ANN001
        for of in range(NE):
            w2t = wpool.tile([P, NF * P], BF16, name="w2t", tag="w2")
            nc.sync.dma_start(w2t[:], d["w2"][of])
            ops = ps_tile("ops", "mm", 3)
            for fb in range(NF):
                nc.tensor.matmul(
                    ops[:], w2t[:, fb * P:fb * P + P],
                    hft[:, fb * 512:fb * 512 + 512],
                    start=(fb == 0), stop=(fb == NF - 1))
            o = of * RO + t * 512
            ot = outp.tile([P, 512], F32, name="ot", tag="ot")
            nc.vector.tensor_add(ot[:], ops[:], h2b[:, o:o + 512])
            nc.sync.dma_start(d["out_t"][:, o:o + 512], ot[:])

    ff1_half(xn3_0, hft0, w1p)
    ff2_half(hft0, 0, w2p)
    ff1_half(xn3_1, hft1, w1p)
    ff2_half(hft1, 1, w2p)

    # teardown (reverse alloc order per side)
    w2p.release()
    outp.release()
    w1p.release()
    hft1p.release()
    hft0p.release()
    h1bp.release()
    xn3p.release()
    # stats3 is under xn3p? alloc order: stats3p, xn3p, wo2p (wo2p released)
    stats3p.release()
    h2bp.release()
    sq_pool.release()
    tmp.release()
    consts.release()
    ps.release()


# ---------------------------------------------------------------------------
# host side: input swizzling, weight folding, output assembly
# ---------------------------------------------------------------------------

def _swz_w(w_t):
    """[E_in, N] (already [in, out]) -> SBUF image [P, (E_in/P)*N]."""
    e_in, n = w_t.shape
    return np.ascontiguousarray(
        w_t.reshape(e_in // P, P, n).transpose(1, 0, 2).reshape(P, -1))


def _swz_blk(w_t):
    """[E_in, N] -> block-streamed [N/P, P, (E_in/P)*P] (fout-block major)."""
    e_in, n = w_t.shape
    return np.ascontiguousarray(
        w_t.reshape(e_in // P, P, n // P, P).transpose(2, 1, 0, 3)
        .reshape(n // P, P, (e_in // P) * P))


def _chunk_groups(rc, h):
    """group order within 512-row chunk rc for core-half h (own first)."""
    return [4 * rc + h, 4 * rc + 2 + h, 4 * rc + 1 - h, 4 * rc + 3 - h]


def _own_rows(h):
    """owned rows in q/attn/output column order (rc-major, 2 groups each)."""
    idx = []
    for rc in range(4):
        for g in (4 * rc + h, 4 * rc + 2 + h):
            idx.extend(range(g * P, (g + 1) * P))
    return np.array(idx)


def _key_rows(h):
    """key rows in k1/v1 column order (permuted chunks)."""
    idx = []
    for rc in range(4):
        for g in _chunk_groups(rc, h):
            idx.extend(range(g * P, (g + 1) * P))
    return np.array(idx)


def make_in_maps(inputs):
    f32 = np.float32
    tgt = np.asarray(inputs["tgt_embs"], f32)
    src = np.asarray(inputs["src_encs"], f32)

    g1 = np.asarray(inputs["ln1_g"], f32)
    g2 = np.asarray(inputs["ln2_g"], f32)
    g3 = np.asarray(inputs["ln3_g"], f32)
    for nm in ("sa_bq", "sa_bk", "sa_bv", "sa_bo", "ca_bq", "ca_bk", "ca_bv",
               "ca_bo", "ff_b1", "ff_b2", "ln1_b", "ln2_b", "ln3_b"):
        assert np.abs(np.asarray(inputs[nm])).max() == 0.0, \
            f"nonzero bias {nm} not supported"

    # qkv weights go to fp8 at x32 so they sit in e4m3's normal range; the
    # 1/sqrt(E) score scale and the x32 factors are undone on-chip (exp
    # scale=1/32768 for q*k, x1/32 on the attention-output copies)
    F8H = ml_dtypes.float8_e4m3

    def to8(x):
        return np.clip(x * 32.0, -240.0, 240.0).astype(F8H)

    wq1 = np.asarray(inputs["sa_Wq"], f32) * g1[None, :]
    wk1 = np.asarray(inputs["sa_Wk"], f32) * g1[None, :]
    wv1 = np.asarray(inputs["sa_Wv"], f32) * g1[None, :]
    wo1 = np.asarray(inputs["sa_Wo"], f32)
    wq2 = np.asarray(inputs["ca_Wq"], f32) * g2[None, :]
    wk2 = np.asarray(inputs["ca_Wk"], f32)
    wv2 = np.asarray(inputs["ca_Wv"], f32)
    wo2 = np.asarray(inputs["ca_Wo"], f32)
    w1 = np.asarray(inputs["ff_W1"], f32) * g3[None, :]
    w2 = np.asarray(inputs["ff_W2"], f32)

    w_common = {
        "wv1": _swz_w(to8(wv1.T)).reshape(P, NE, E),
        "wv2": _swz_w(to8(wv2.T)).reshape(P, NE, E),
        "wq1": _swz_blk(to8(wq1.T)).reshape(NE, P, NE, P),
        "wk1": _swz_blk(to8(wk1.T)).reshape(NE, P, NE, P),
        "wq2": _swz_blk(to8(wq2.T)).reshape(NE, P, NE, P),
        "wk2": _swz_blk(to8(wk2.T)).reshape(NE, P, NE, P),
        "wo1": _swz_blk(wo1.T.astype(BF)),
        "wo2": _swz_blk(wo2.T.astype(BF)),
        "w1": _swz_blk(w1.T.astype(BF)),
        "w2": _swz_blk(w2.T.astype(BF)),
    }

    in_maps = []
    for c in range(NCORES):
        b, h = c // 2, c % 2
        krows = _key_rows(h)
        qrows = _own_rows(h)
        # tgt permuted chunk-major [4, P, NE, 512]
        tgt_perm = tgt[b][krows].T.astype(BF)  # [E, S] in key order
        tgt_t = np.ascontiguousarray(
            tgt_perm.reshape(NE, P, 4, 512).transpose(2, 1, 0, 3))
        tgto = np.ascontiguousarray(
            tgt[b][qrows].T.reshape(NE, P, 2, 512)
            .transpose(2, 1, 0, 3).reshape(2, P, NE * 512))
        # src natural order, 256-col chunks [8, P, NE, 256], fp8 x1
        src_t = np.ascontiguousarray(
            np.clip(src[b].T, -240, 240).astype(F8H)
            .reshape(NE, P, 8, 256).transpose(2, 1, 0, 3))
        # causal masks: key rows kr vs query rows qg (both permuted orders)
        mask = np.zeros((2, 8, P, 512), np.float32)
        for t in range(2):
            qg = qrows[t * 512:(t + 1) * 512]
            for kb in range(8):
                kr = krows[(8 * t + kb) * P:(8 * t + kb + 1) * P]
                mask[t, kb] = np.where(kr[:, None] <= qg[None, :], 0.0, NEG)
        in_maps.append({
            "tgt_t": tgt_t,
            "tgto": tgto,
            "src_t": src_t,
            "mask": mask.astype(BF),
            **w_common,
        })
    return in_maps


def assemble_output(results):
    out = np.empty((B, S, E), np.float32)
    for c in range(NCORES):
        b, h = c // 2, c % 2
        arr = np.asarray(results[c]["out_t"])  # [P, NE*RO]
        a = arr.reshape(P, NE, 8, P).transpose(2, 3, 1, 0).reshape(8, P, E)
        qrows = _own_rows(h)
        for j in range(8):
            out[b, qrows[j * P:(j + 1) * P], :] = a[j]
    return out


def get_nc():
    if "nc" not in _NC_CACHE:
        _NC_CACHE["nc"] = _build_program()
    return _NC_CACHE["nc"]


def _axon_reset():
    """Recover a wedged remote NeuronCore (NRT_EXEC_UNIT_UNRECOVERABLE)."""
    try:
        import ctypes
        lib = ctypes.CDLL("/opt/axon/libaxon_pjrt.so")
        lib.axon_reset.restype = ctypes.c_int64
        lib.axon_reset()
    except Exception:
        pass


def kernel(**inputs):
    global LAST_RESULTS
    in_maps = make_in_maps(inputs)
    nc = get_nc()
    last_err = None
    for attempt in range(3):
        try:
            res = run_bass_kernel_spmd(nc, in_maps, list(range(NCORES)))
            break
        except Exception as e:  # wedged device -> reset + retry
            last_err = e
            _axon_reset()
    else:
        raise last_err
    LAST_RESULTS = res
    return assemble_output(res.results)


# revision 73
# speedup vs baseline: 1.1000x; 1.0473x over previous
"""Trainium2 Bass kernel for a single-head transformer decoder layer.

Model (per batch element, S=2048, E=1024, F=4096):
    xn  = LN(tgt);  sa = causal_attn(xn)       ; h1  = tgt + sa
    xn2 = LN(h1);   ca = cross_attn(xn2, src)  ; h2  = h1 + ca
    xn3 = LN(h2);   ff = relu(xn3@W1.T)@W2.T   ; out = h2 + ff

Sharding: 8 cores = 4 batches x 2-way query-row split.  Core c owns batch
b=c//2 and, within each 512-row chunk rc, the two interleaved 128-row
groups {4rc+h, 4rc+2+h} (h=c%2, zig-zag so causal work is balanced).
The host PERMUTES each 512-row chunk so the core's own 256 rows come
first — the program is identical on every core; only DMA'd data differs.
That makes Q projection read the already-LN'd xn chunk tiles directly
(owned cols = first 256 of each chunk), eliminating the v1 kernel's
separate owned-row LN pass.  K/V over all 2048 rows are duplicated within
each pair; no collectives.

On-chip layout: activations stored transposed [feature(part), row(free)];
every matmul runs without on-chip transposes.  LN gains are folded into
weights on the host; biases are all zero.

fp8 (TRN e4m3) with DoubleRow perf mode on all six Q/K/V projections:
LN outputs (xn/xn2) and src are quantized to fp8; q/k/v weights are
host-scaled x32 into e4m3's normal range.  The x32 scales and the
1/sqrt(E) score scale are undone for free — q,k stay x32 so scores come
out x32768, folded into the softmax exp's scale; v stays x32, folded
into the attention-output PSUM copy (x1/32).  Scores/AV/O-proj/FFN stay
bf16: measured on HW, going fp8 there too reaches 1.7e-2 rel err (vs
the 2e-2 gate) for only ~15% more speed — not worth the risk; this
config measures 9.6e-3 (2x margin).

Scheduling: the Tile scheduler is a per-engine priority-heap list
scheduler (priority = emission order), so critical-path work is emitted
first and independent filler GEMMs (K2/V2 from src) are interleaved
into the O1/LN2/Q2 chain emissions to fill PE bubbles and keep the HAM
clock warm.  Weight blocks are streamed (block-major DRAM layouts).
All ACT functions are pinned to one LUT table set (see below) — the
default chooser reloads tables (1.28us) between ln/exp ops otherwise.
rsqrt and 1/x are computed as exp(-0.5*ln(x)) / exp(-ln(x)) on ACT:
ln+exp live in the same table set and avoid the slow DVE-iterative
reciprocal (3.4us) on the LN/softmax chains.

Numerics: PSUM fp32, LN stats + softmax sums fp32 (A/B stat tiles
bf16), residual stream bf16 (adds in fp32), output fp32.
Measured: ~850us/core (worst ~880k ns), rel err 9.627e-03.
"""

import os
import sys

import numpy as np

for _p in ("/opt/trn_rl_repo", "/root/.axon_site/_ro/trn_rl_repo"):
    if os.path.isdir(_p) and _p not in sys.path:
        sys.path.insert(0, _p)

import ml_dtypes  # noqa: E402

import concourse.bass as bass  # noqa: E402
import concourse.tile as tile  # noqa: E402
from concourse import bacc, mybir  # noqa: E402
from concourse.bass_utils import run_bass_kernel_spmd  # noqa: E402

# Pin every ACT function to the one table set that holds all of ours
# (exp, ln, square, copy, relu, identity).  The default per-instruction
# chooser assigns exp->exp_and_others but ln->natural_log, which makes the
# ACT engine reload its LUT (1.28us) between almost every pair of ops in
# LN/softmax chains.  Emptying the other sets (indices preserved, so the
# emitted act_func_set_id still matches the canonical act_info.json)
# forces one table for the whole kernel: zero reloads after startup.
import concourse.hw_specs as _hw_specs  # noqa: E402
import concourse.bass_interp as _interp  # noqa: E402

_ORIG_GAT = _hw_specs.get_activation_tables
_KEEP_SET = "natural_log_exp_and_others"


def _pinned_activation_tables(arch):
    tabs = _ORIG_GAT(arch)
    return {k: (v if k == _KEEP_SET else set()) for k, v in tabs.items()}



E = 1024
S = 2048
B = 4
F = 4096
P = 128
NE = E // P          # 8 feature blocks
NF = F // P          # 32 ff blocks
NKB = S // P         # 16 key-row blocks
RO = 1024            # owned query rows per core
NCORES = 8

F32 = mybir.dt.float32
BF16 = mybir.dt.bfloat16
F8 = mybir.dt.float8e4        # TRN E4M3: max +-240, matches ml_dtypes e4m3
BF = ml_dtypes.bfloat16
ALU = mybir.AluOpType
ACT_F = mybir.ActivationFunctionType
DR = mybir.MatmulPerfMode.DoubleRow

NEG = -1e30

_NC_CACHE = {}
LAST_RESULTS = None  # BassKernelResults of the most recent hardware run


def _build_program():
    """Emit the single SPMD program (identical for all 8 cores)."""
    bacc.get_activation_tables = _pinned_activation_tables
    _interp.get_activation_tables = _pinned_activation_tables
    try:
        return _build_program_inner()
    finally:
        bacc.get_activation_tables = _ORIG_GAT
        _interp.get_activation_tables = _ORIG_GAT


def _build_program_inner():
    nc = bacc.Bacc(
        "TRN2",
        target_bir_lowering=False,
        debug=False,
        enable_asserts=False,
        num_devices=NCORES,
    )

    d = {}
    # tgt, permuted per 512-chunk (own rows first), chunk-major
    d["tgt_t"] = nc.dram_tensor("tgt_t", [4, P, NE, 512], BF16,
                                kind="ExternalInput")
    # raw tgt, owned rows only, t-half major (residual stream seed)
    d["tgto"] = nc.dram_tensor("tgto", [2, P, NE * 512], F32,
                               kind="ExternalInput")
    # src, natural order, 256-col chunks for streaming (fp8, x1 scale)
    d["src_t"] = nc.dram_tensor("src_t", [8, P, NE, 256], F8,
                                kind="ExternalInput")
    d["mask"] = nc.dram_tensor("mask", [2, 8, P, 512], BF16, kind="ExternalInput")
    # full-resident moving-side qkv weights (fp8, x32 scale)
    for w in ("wv1", "wv2"):
        d[w] = nc.dram_tensor(w, [P, NE, E], F8, kind="ExternalInput")
    # block-streamed stationary-side qkv weights (fp8, x32 scale)
    for w in ("wq1", "wk1", "wq2", "wk2"):
        d[w] = nc.dram_tensor(w, [NE, P, NE, P], F8, kind="ExternalInput")
    # block-streamed o-proj weights (bf16)
    for w in ("wo1", "wo2"):
        d[w] = nc.dram_tensor(w, [NE, P, NE * P], BF16, kind="ExternalInput")
    d["w1"] = nc.dram_tensor("w1", [NF, P, NE * P], BF16, kind="ExternalInput")
    d["w2"] = nc.dram_tensor("w2", [NE, P, NF * P], BF16, kind="ExternalInput")
    d["out_t"] = nc.dram_tensor("out_t", [P, NE * RO], F32, kind="ExternalOutput")

    with tile.TileContext(nc) as tc:
        _emit(tc, {k: v.ap() for k, v in d.items()})

    nc.compile()
    return nc


def _emit(tc, d):
    nc = tc.nc

    def pool(name, bufs=1, side="left"):
        return tc.alloc_tile_pool(name=name, bufs=bufs, side=side)

    # one PSUM pool; per-tag bufs; total = 8 banks
    ps = tc.alloc_tile_pool(name="ps", bufs=1, space="PSUM")

    def ps_tile(name, tag, bufs, shape=(P, 512)):
        return ps.tile(list(shape), F32, name=name, tag=tag, bufs=bufs)

    # ---------------- persistent small pools (right) -----------------------
    consts = pool("consts", side="right")
    ones_f = consts.tile([P, P], F32, name="ones_f", tag="ones_f")
    nc.vector.memset(ones_f[:], 1.0)
    ones_b = consts.tile([P, P], BF16, name="ones_b", tag="ones_b")
    nc.vector.memset(ones_b[:], 1.0)
    ones_8 = consts.tile([P, 1], F8, name="ones_8", tag="ones_8")
    nc.vector.memset(ones_8[:], 1.0)
    eps_t = consts.tile([P, 1], F32, name="eps_t", tag="eps")
    nc.vector.memset(eps_t[:], 1e-5)

    tmp = pool("tmp", bufs=1, side="right")
    sq_pool = pool("sq", bufs=2, side="right")

    # ---------------- long-lived left pools --------------------------------
    h1bp = pool("h1bp", side="left")
    h1b = h1bp.tile([P, NE * RO], BF16, name="h1b", tag="h1b")
    attnp = pool("attnp", bufs=1, side="left")      # attn accum (2 slots)
    etp = pool("etp", bufs=1, side="left")          # exp(scores), 16 KB
    invp = pool("invp", side="left")        # softmax inv + sm_sb

    k1p = pool("k1p")
    k1 = k1p.tile([P, NE, S], BF16, name="k1", tag="k1")
    v1p = pool("v1p")
    v1 = v1p.tile([P, NKB, E], BF16, name="v1", tag="v1")
    q1p = pool("q1p")
    q1 = q1p.tile([P, NE, RO], BF16, name="q1", tag="q1")

    # ---------------- right: front streams ---------------------------------
    maskp = pool("maskp", bufs=3, side="right")
    statsp = pool("statsp", side="right")
    in_pool = pool("inp", bufs=3, side="right")   # whole-chunk tiles
    xnfp = pool("xnf", bufs=3, side="right")
    wv1p = pool("wv1p", side="right")
    wv1 = wv1p.tile([P, NE, E], F8, name="wv1_sb", tag="wv")
    wk1p = pool("wk1p", bufs=3, side="right")
    wq1p = pool("wq1p", bufs=3, side="right")

    # ---------------- helpers ----------------------------------------------
    def ln_stats(get_tile, rc, stat_pool, prefix):
        """Column stats for one 512-col chunk of transposed activations.

        get_tile(eb) -> [P, 512] bf16 AP.  Returns (A, Bv): [P,512] bf16
        broadcast along partitions: A = rsqrt(var+eps), Bv = mean * A.
        """
        sum_x = ps_tile(f"{prefix}sx", "sx", 1)
        sum_xx = ps_tile(f"{prefix}sxx", "sxx", 1)
        for eb in range(NE):
            t = get_tile(eb)
            sq = sq_pool.tile([P, 512], BF16, name=f"{prefix}sq", tag="sq")
            nc.scalar.square(sq[:], t)
            nc.tensor.matmul(sum_x[:], ones_b[:], t,
                             start=(eb == 0), stop=(eb == NE - 1))
            nc.tensor.matmul(sum_xx[:], ones_b[:], sq[:],
                             start=(eb == 0), stop=(eb == NE - 1))
        mu = tmp.tile([P, 512], F32, name=f"{prefix}mu", tag="t0")
        nc.vector.tensor_scalar_mul(mu[:], sum_x[:], 1.0 / E)
        var = tmp.tile([P, 512], F32, name=f"{prefix}var", tag="t2")
        nc.vector.tensor_mul(var[:], mu[:], mu[:])
        nc.vector.scalar_tensor_tensor(
            var[:], sum_xx[:], 1.0 / E, var[:], ALU.mult, ALU.subtract)
        # rsqrt(var+eps) = exp(-0.5*ln(var+eps)) — ln and exp live in the
        # same ACT table set, so the kernel never reloads tables and no
        # (slow, DVE-iterative) reciprocal is needed
        lnv = tmp.tile([P, 512], F32, name=f"{prefix}lnv", tag="t3")
        nc.scalar.activation(lnv[:], var[:], ACT_F.Ln, bias=eps_t[:])
        a = stat_pool.tile([P, 512], BF16, name=f"{prefix}A{rc}",
                           tag=f"A{rc % 2}")
        nc.scalar.activation(a[:], lnv[:], ACT_F.Exp, scale=-0.5)
        bv = stat_pool.tile([P, 512], BF16, name=f"{prefix}B{rc}",
                            tag=f"B{rc % 2}")
        with nc.allow_low_precision(reason="bf16 LN stats, error ~2^-9"):
            nc.vector.tensor_mul(bv[:], mu[:], a[:])
        return a, bv

    def ln_apply(dst, src_ap, a, bv, prefix):
        """dst (bf16) = src*A - Bv (all-bf16 SBUF ops -> fast DVE mode)."""
        t = tmp.tile([P, 512], BF16, name=f"{prefix}ap", tag="t0b")
        with nc.allow_low_precision(reason="bf16 LN apply, error ~2^-9"):
            nc.vector.tensor_mul(t[:], src_ap, a[:])
            nc.vector.tensor_sub(dst, t[:], bv[:])

    def wblk_dma(wpool, wname, fb):
        t = wpool.tile([P, NE * P], BF16, name=f"{wname}t", tag="w")
        nc.sync.dma_start(t[:], d[wname][fb])
        return t

    def wblk8_dma(wpool, wname, fb):
        t = wpool.tile([P, NE, P], F8, name=f"{wname}t", tag="w")
        nc.sync.dma_start(t[:], d[wname][fb])
        return t

    # ---------------- front: per-chunk LN1 -> Q1/K1/V1 ---------------------
    def front_chunk(rc, post_dma=None):
        ct = in_pool.tile([P, NE, 512], BF16, name="tgt_in", tag="xin")
        nc.sync.dma_start(ct[:], d["tgt_t"][rc])
        if post_dma is not None:
            post_dma()
        a, bv = ln_stats(lambda eb: ct[:, eb, :], rc, statsp, f"l1c{rc}")
        xn = xnfp.tile([P, NE, 512], F8, name="xn_rc", tag="xn")
        for eb in range(NE):
            ln_apply(xn[:, eb, :], ct[:, eb, :], a, bv, f"l1c{rc}")
        # Q1 over owned 256 cols (front half of the permuted chunk)
        for fblk in range(NE):
            wq1t = wblk8_dma(wq1p, "wq1", fblk)
            qp = ps_tile("q1ps", "mm", 3, shape=(P, 256))
            for q in range(NE // 2):
                nc.tensor.matmul(
                    qp[:], wq1t[:, 2 * q:2 * q + 2, :],
                    xn[:, 2 * q:2 * q + 2, 0:256],
                    start=(q == 0), stop=(q == NE // 2 - 1), perf_mode=DR)
            nc.scalar.copy(q1[:, fblk, rc * 256:rc * 256 + 256], qp[:])
        # K1: all 512 cols
        for kf in range(NE):
            wk1t = wblk8_dma(wk1p, "wk1", kf)
            kp = ps_tile("kp", "mm", 3)
            for q in range(NE // 2):
                nc.tensor.matmul(
                    kp[:], wk1t[:, 2 * q:2 * q + 2, :],
                    xn[:, 2 * q:2 * q + 2, :],
                    start=(q == 0), stop=(q == NE // 2 - 1), perf_mode=DR)
            nc.scalar.copy(k1[:, kf, rc * 512:rc * 512 + 512], kp[:])
        # V1: natural layout [rows, feat]
        for rb in range(4):
            for vf in range(2):
                vp = ps_tile("vp", "mm", 3)
                for q in range(NE // 2):
                    nc.tensor.matmul(
                        vp[:],
                        xn[:, 2 * q:2 * q + 2, rb * P:rb * P + P],
                        wv1[:, 2 * q:2 * q + 2, vf * 512:vf * 512 + 512],
                        start=(q == 0), stop=(q == NE // 2 - 1), perf_mode=DR)
                nc.scalar.copy(v1[:, rc * 4 + rb, vf * 512:vf * 512 + 512],
                               vp[:])

    # ---------------- attention half ---------------------------------------
    def attn_half(q_sb, k_sb, v_sb, ext, masked, t, attn, prefix):
        """scores+softmax+AV for one 512-query half; returns inv tile.

        q/k/v/et are fp8; scores and AV run in DoubleRow mode (two
        128-row contraction blocks per matmul).
        """
        et = etp.tile([P, NKB, 512], BF16, name=f"{prefix}et", tag="et")
        for kb in range(ext):
            sp = ps_tile(f"{prefix}sp", "mm", 3)
            for eb in range(NE):
                nc.tensor.matmul(
                    sp[:],
                    k_sb[:, eb, kb * P:kb * P + P],
                    q_sb[:, eb, t * 512:t * 512 + 512],
                    start=(eb == 0), stop=(eb == NE - 1))
            if masked and kb >= 8 * t:
                mt = maskp.tile([P, 512], BF16, name=f"{prefix}mt", tag="mt")
                nc.sync.dma_start(mt[:], d["mask"][t, kb - 8 * t])
                nc.vector.tensor_add(sp[:], sp[:], mt[:])
            # q,k carry x32 weight scale and the 1/sqrt(E) fold moves here:
            # scores are x(32*32*32) -> exp scale 1/32768
            nc.scalar.activation(et[:, kb, :], sp[:], ACT_F.Exp,
                                 scale=1.0 / 32768)
        # AV first (doesn't need the denominator), softmax sum after — the
        # inv chain then overlaps the o_proj matmuls instead of gating them
        for af in range(NE):
            ap_ = ps_tile(f"{prefix}avp", "av", 2)
            for kb in range(ext):
                nc.tensor.matmul(
                    ap_[:],
                    v_sb[:, kb, af * P:af * P + P],
                    et[:, kb, :],
                    start=(kb == 0), stop=(kb == ext - 1))
            o = af * RO + t * 512
            # v carries the x32 weight scale -> undo it here
            nc.scalar.mul(attn[:, o:o + 512], ap_[:], 1.0 / 32)
        # softmax denominator via ones-matmul column sums (sm and its
        # broadcast share one PSUM bank — they are strictly sequential)
        sm = ps_tile(f"{prefix}sm", "sm", 1, shape=(1, 512))
        for kb in range(ext):
            nc.tensor.matmul(sm[:], ones_b[:, 0:1], et[:, kb, :],
                             start=(kb == 0), stop=(kb == ext - 1))
        sm_sb = invp.tile([1, 512], F32, name=f"{prefix}smsb", tag="smsb")
        nc.scalar.copy(sm_sb[:], sm[:])
        sb_ps = ps_tile(f"{prefix}smb", "sm", 1)
        nc.tensor.matmul(sb_ps[:], ones_f[0:1, :], sm_sb[:],
                         start=True, stop=True)
        # 1/x = exp(-ln(x)) — same ACT table set as everything else
        lns = tmp.tile([P, 512], F32, name=f"{prefix}lns", tag="t3")
        nc.scalar.activation(lns[:], sb_ps[:], ACT_F.Ln)
        inv = invp.tile([P, 512], BF16, name=f"{prefix}inv{t}",
                        tag=f"inv{t}")
        nc.scalar.activation(inv[:], lns[:], ACT_F.Exp, scale=-1.0)
        return inv

    def o_proj_half(attn, inv, wpool, wname, res_fn, dst, t, tag):
        """dst[:, t-half] (bf16) = (W_o.T @ attn) * inv + residual."""
        for of in range(NE):
            wt = wblk_dma(wpool, wname, of)
            op = ps_tile(f"{tag}op", "mm", 3)
            for ab in range(NE):
                nc.tensor.matmul(
                    op[:], wt[:, ab * P:ab * P + P],
                    attn[:, ab * RO + t * 512:ab * RO + t * 512 + 512],
                    start=(ab == 0), stop=(ab == NE - 1))
            cp = tmp.tile([P, 512], BF16, name=f"{tag}cp", tag="t1b")
            nc.scalar.copy(cp[:], op[:])
            t1 = tmp.tile([P, 512], BF16, name=f"{tag}on", tag="t2b")
            with nc.allow_low_precision(reason="bf16 attn out, ~2^-9"):
                nc.vector.tensor_mul(t1[:], cp[:], inv[:])
                o = of * RO + t * 512
                nc.vector.tensor_add(dst[:, o:o + 512], t1[:], res_fn(of))

    # ======================= emission =======================================
    attn1 = attnp.tile([P, NE * RO], BF16, name="attn1", tag="attn")

    # wv1 DMA after chunk 0's tiles so the first LN stats aren't delayed
    front_chunk(0, post_dma=lambda: nc.sync.dma_start(wv1[:], d["wv1"][:]))
    front_chunk(1)
    inv_sa0 = attn_half(q1, k1, v1, 8, True, 0, attn1, "sa0")
    front_chunk(2)
    front_chunk(3)
    inv_sa1 = attn_half(q1, k1, v1, 16, True, 1, attn1, "sa1")

    # release front pools (reverse alloc order per side)
    wq1p.release()
    wk1p.release()
    wv1p.release()
    xnfp.release()
    in_pool.release()
    statsp.release()
    maskp.release()
    q1p.release()
    v1p.release()
    k1p.release()

    # mid-phase pools (right), in freed front space
    stats2p = pool("stats2", side="right")
    srcs = pool("srcs", bufs=4, side="right")
    wv2p = pool("wv2p", side="right")
    wv2 = wv2p.tile([P, NE, E], F8, name="wv2_sb", tag="wv2")
    nc.sync.dma_start(wv2[:], d["wv2"][:])
    wk2p = pool("wk2p", bufs=2, side="right")
    tgtohp = pool("tgtohp", side="right")
    wq2p = pool("wq2p", bufs=2, side="right")
    wo1p = pool("wo1p", bufs=3, side="right")
    xn2p = pool("xn2p", side="right")

    def tgto_half(t):
        th = tgtohp.tile([P, NE * 512], F32, name=f"tgto{t}", tag="tgto")
        nc.sync.dma_start(th[:], d["tgto"][t])
        return th

    # left: q2/k2/v2 destination tiles (before the o1/ln2 chain, so the
    # interleaved K2/V2 filler blocks can write them)
    q2p = pool("q2p")
    q2 = q2p.tile([P, NE, RO], BF16, name="q2", tag="q2")
    k2p = pool("k2p")
    k2 = k2p.tile([P, NE, S], BF16, name="k2", tag="k2")
    v2p = pool("v2p")
    v2 = v2p.tile([P, NKB, E], BF16, name="v2", tag="v2")

    def k2v2_block(hp):
        """K2/V2 over one 512-row src block (2x256 sub-chunks), streamed."""
        stiles = []
        for sc in range(2):
            t = srcs.tile([P, NE, 256], F8, name="src_in", tag="src")
            nc.sync.dma_start(t[:], d["src_t"][2 * hp + sc])
            stiles.append(t)
        for kf in range(NE):
            wk2t = wblk8_dma(wk2p, "wk2", kf)
            for sc in range(2):
                kp = ps_tile("kp2", "mm", 3, shape=(P, 256))
                for q in range(NE // 2):
                    nc.tensor.matmul(
                        kp[:], wk2t[:, 2 * q:2 * q + 2, :],
                        stiles[sc][:, 2 * q:2 * q + 2, :],
                        start=(q == 0), stop=(q == NE // 2 - 1), perf_mode=DR)
                o = (2 * hp + sc) * 256
                nc.scalar.copy(k2[:, kf, o:o + 256], kp[:])
        for sc in range(2):
            for rb in range(2):
                for vf in range(2):
                    vp = ps_tile("vp2", "mm", 3)
                    for q in range(NE // 2):
                        nc.tensor.matmul(
                            vp[:],
                            stiles[sc][:, 2 * q:2 * q + 2, rb * P:rb * P + P],
                            wv2[:, 2 * q:2 * q + 2, vf * 512:vf * 512 + 512],
                            start=(q == 0), stop=(q == NE // 2 - 1),
                            perf_mode=DR)
                    nc.scalar.copy(
                        v2[:, (2 * hp + sc) * 2 + rb, vf * 512:vf * 512 + 512],
                        vp[:])

    def ln2_q2_half(t):
        a, bv = ln_stats(
            lambda eb: h1b[:, eb * RO + t * 512:eb * RO + t * 512 + 512],
            t, stats2p, f"l2t{t}")
        xn2 = xn2p.tile([P, NE, 512], F8, name="xn2", tag="xn2")
        for eb in range(NE):
            ln_apply(xn2[:, eb, :],
                     h1b[:, eb * RO + t * 512:eb * RO + t * 512 + 512],
                     a, bv, f"l2t{t}")
        for fblk in range(NE):
            wq2t = wblk8_dma(wq2p, "wq2", fblk)
            qp = ps_tile("q2ps", "mm", 3)
            for q in range(NE // 2):
                nc.tensor.matmul(
                    qp[:], wq2t[:, 2 * q:2 * q + 2, :],
                    xn2[:, 2 * q:2 * q + 2, :],
                    start=(q == 0), stop=(q == NE // 2 - 1), perf_mode=DR)
            nc.scalar.copy(q2[:, fblk, t * 512:t * 512 + 512], qp[:])

    # critical path interleaved with K2/V2 filler blocks: the scheduler
    # slots the filler MMs into the o_proj/LN2/Q2 dependency-chain bubbles
    th0 = tgto_half(0)
    o_proj_half(attn1, inv_sa0, wo1p, "wo1",
                lambda of: th0[:, of * 512:of * 512 + 512], h1b, 0, "o1a")
    k2v2_block(0)
    ln2_q2_half(0)
    k2v2_block(1)
    th1 = tgto_half(1)
    o_proj_half(attn1, inv_sa1, wo1p, "wo1",
                lambda of: th1[:, of * 512:of * 512 + 512], h1b, 1, "o1b")
    k2v2_block(2)
    ln2_q2_half(1)
    k2v2_block(3)

    # release pools whose last readers are now emitted (LIFO)
    xn2p.release()
    wo1p.release()
    wq2p.release()
    tgtohp.release()
    wk2p.release()
    wv2p.release()
    srcs.release()
    stats2p.release()

    # ca-phase pools
    h2bp = pool("h2bp", side="right")
    h2b = h2bp.tile([P, NE * RO], BF16, name="h2b", tag="h2b")
    stats3p = pool("stats3", side="right")
    xn3p = pool("xn3p", bufs=2, side="right")
    wo2p = pool("wo2p", bufs=2, side="right")

    attn2 = attnp.tile([P, NE * RO], BF16, name="attn2", tag="attn")

    def ln3_half(t):
        a, bv = ln_stats(
            lambda eb: h2b[:, eb * RO + t * 512:eb * RO + t * 512 + 512],
            t, stats3p, f"l3t{t}")
        xn3 = xn3p.tile([P, NE * 512], BF16, name="xn3", tag="xn3")
        for eb in range(NE):
            ln_apply(xn3[:, eb * 512:eb * 512 + 512],
                     h2b[:, eb * RO + t * 512:eb * RO + t * 512 + 512],
                     a, bv, f"l3t{t}")
        return xn3

    inv_ca0 = attn_half(q2, k2, v2, 16, False, 0, attn2, "ca0")
    o_proj_half(attn2, inv_ca0, wo2p, "wo2",
                lambda of: h1b[:, of * RO:of * RO + 512], h2b, 0, "o2a")
    xn3_0 = ln3_half(0)
    inv_ca1 = attn_half(q2, k2, v2, 16, False, 1, attn2, "ca1")
    o_proj_half(attn2, inv_ca1, wo2p, "wo2",
                lambda of: h1b[:, of * RO + 512:of * RO + 1024], h2b, 1,
                "o2b")
    xn3_1 = ln3_half(1)

    # attention no longer needed; free left space for FF hidden tiles
    wo2p.release()
    v2p.release()
    k2p.release()
    q2p.release()
    invp.release()
    etp.release()
    attnp.release()

    hft0p = pool("hft0p")
    hft0 = hft0p.tile([P, NF * 512], BF16, name="hft0", tag="hft0")
    hft1p = pool("hft1p")
    hft1 = hft1p.tile([P, NF * 512], BF16, name="hft1", tag="hft1")
    w1p = pool("w1p", bufs=3, side="right")
    outp = pool("outp", bufs=4, side="right")
    w2p = pool("w2p", bufs=2, side="right")

    def ff1_half(xn3, hft, wpool):
        for fb in range(NF):
            w1t = wpool.tile([P, NE * P], BF16, name="w1t", tag="w1")
            nc.sync.dma_start(w1t[:], d["w1"][fb])
            hps = ps_tile("hps", "mm", 3)
            for eb in range(NE):
                nc.tensor.matmul(
                    hps[:], w1t[:, eb * P:eb * P + P],
                    xn3[:, eb * 512:eb * 512 + 512],
                    start=(eb == 0), stop=(eb == NE - 1))
            nc.scalar.activation(hft[:, fb * 512:fb * 512 + 512], hps[:],
                                 ACT_F.Relu)

    def ff2_half(hft, t, wpool):  # noqa: ANN001
        for of in range(NE):
            w2t = wpool.tile([P, NF * P], BF16, name="w2t", tag="w2")
            nc.sync.dma_start(w2t[:], d["w2"][of])
            ops = ps_tile("ops", "mm", 3)
            for fb in range(NF):
                nc.tensor.matmul(
                    ops[:], w2t[:, fb * P:fb * P + P],
                    hft[:, fb * 512:fb * 512 + 512],
                    start=(fb == 0), stop=(fb == NF - 1))
            o = of * RO + t * 512
            ot = outp.tile([P, 512], F32, name="ot", tag="ot")
            nc.vector.tensor_add(ot[:], ops[:], h2b[:, o:o + 512])
            nc.sync.dma_start(d["out_t"][:, o:o + 512], ot[:])

    ff1_half(xn3_0, hft0, w1p)
    ff2_half(hft0, 0, w2p)
    ff1_half(xn3_1, hft1, w1p)
    ff2_half(hft1, 1, w2p)

    # teardown (reverse alloc order per side)
    w2p.release()
    outp.release()
    w1p.release()
    hft1p.release()
    hft0p.release()
    h1bp.release()
    xn3p.release()
    # stats3 is under xn3p? alloc order: stats3p, xn3p, wo2p (wo2p released)
    stats3p.release()
    h2bp.release()
    sq_pool.release()
    tmp.release()
    consts.release()
    ps.release()


# ---------------------------------------------------------------------------
# host side: input swizzling, weight folding, output assembly
# ---------------------------------------------------------------------------

def _swz_w(w_t):
    """[E_in, N] (already [in, out]) -> SBUF image [P, (E_in/P)*N]."""
    e_in, n = w_t.shape
    return np.ascontiguousarray(
        w_t.reshape(e_in // P, P, n).transpose(1, 0, 2).reshape(P, -1))


def _swz_blk(w_t):
    """[E_in, N] -> block-streamed [N/P, P, (E_in/P)*P] (fout-block major)."""
    e_in, n = w_t.shape
    return np.ascontiguousarray(
        w_t.reshape(e_in // P, P, n // P, P).transpose(2, 1, 0, 3)
        .reshape(n // P, P, (e_in // P) * P))


def _chunk_groups(rc, h):
    """group order within 512-row chunk rc for core-half h (own first)."""
    return [4 * rc + h, 4 * rc + 2 + h, 4 * rc + 1 - h, 4 * rc + 3 - h]


def _own_rows(h):
    """owned rows in q/attn/output column order (rc-major, 2 groups each)."""
    idx = []
    for rc in range(4):
        for g in (4 * rc + h, 4 * rc + 2 + h):
            idx.extend(range(g * P, (g + 1) * P))
    return np.array(idx)


def _key_rows(h):
    """key rows in k1/v1 column order (permuted chunks)."""
    idx = []
    for rc in range(4):
        for g in _chunk_groups(rc, h):
            idx.extend(range(g * P, (g + 1) * P))
    return np.array(idx)


def make_in_maps(inputs):
    f32 = np.float32
    tgt = np.asarray(inputs["tgt_embs"], f32)
    src = np.asarray(inputs["src_encs"], f32)

    g1 = np.asarray(inputs["ln1_g"], f32)
    g2 = np.asarray(inputs["ln2_g"], f32)
    g3 = np.asarray(inputs["ln3_g"], f32)
    for nm in ("sa_bq", "sa_bk", "sa_bv", "sa_bo", "ca_bq", "ca_bk", "ca_bv",
               "ca_bo", "ff_b1", "ff_b2", "ln1_b", "ln2_b", "ln3_b"):
        assert np.abs(np.asarray(inputs[nm])).max() == 0.0, \
            f"nonzero bias {nm} not supported"

    # qkv weights go to fp8 at x32 so they sit in e4m3's normal range; the
    # 1/sqrt(E) score scale and the x32 factors are undone on-chip (exp
    # scale=1/32768 for q*k, x1/32 on the attention-output copies)
    F8H = ml_dtypes.float8_e4m3

    def to8(x):
        return np.clip(x * 32.0, -240.0, 240.0).astype(F8H)

    wq1 = np.asarray(inputs["sa_Wq"], f32) * g1[None, :]
    wk1 = np.asarray(inputs["sa_Wk"], f32) * g1[None, :]
    wv1 = np.asarray(inputs["sa_Wv"], f32) * g1[None, :]
    wo1 = np.asarray(inputs["sa_Wo"], f32)
    wq2 = np.asarray(inputs["ca_Wq"], f32) * g2[None, :]
    wk2 = np.asarray(inputs["ca_Wk"], f32)
    wv2 = np.asarray(inputs["ca_Wv"], f32)
    wo2 = np.asarray(inputs["ca_Wo"], f32)
    w1 = np.asarray(inputs["ff_W1"], f32) * g3[None, :]
    w2 = np.asarray(inputs["ff_W2"], f32)

    w_common = {
        "wv1": _swz_w(to8(wv1.T)).reshape(P, NE, E),
        "wv2": _swz_w(to8(wv2.T)).reshape(P, NE, E),
        "wq1": _swz_blk(to8(wq1.T)).reshape(NE, P, NE, P),
        "wk1": _swz_blk(to8(wk1.T)).reshape(NE, P, NE, P),
        "wq2": _swz_blk(to8(wq2.T)).reshape(NE, P, NE, P),
        "wk2": _swz_blk(to8(wk2.T)).reshape(NE, P, NE, P),
        "wo1": _swz_blk(wo1.T.astype(BF)),
        "wo2": _swz_blk(wo2.T.astype(BF)),
        "w1": _swz_blk(w1.T.astype(BF)),
        "w2": _swz_blk(w2.T.astype(BF)),
    }

    in_maps = []
    for c in range(NCORES):
        b, h = c // 2, c % 2
        krows = _key_rows(h)
        qrows = _own_rows(h)
        # tgt permuted chunk-major [4, P, NE, 512]
        tgt_perm = tgt[b][krows].T.astype(BF)  # [E, S] in key order
        tgt_t = np.ascontiguousarray(
            tgt_perm.reshape(NE, P, 4, 512).transpose(2, 1, 0, 3))
        tgto = np.ascontiguousarray(
            tgt[b][qrows].T.reshape(NE, P, 2, 512)
            .transpose(2, 1, 0, 3).reshape(2, P, NE * 512))
        # src natural order, 256-col chunks [8, P, NE, 256], fp8 x1
        src_t = np.ascontiguousarray(
            np.clip(src[b].T, -240, 240).astype(F8H)
            .reshape(NE, P, 8, 256).transpose(2, 1, 0, 3))
        # causal masks: key rows kr vs query rows qg (both permuted orders)
        mask = np.zeros((2, 8, P, 512), np.float32)
        for t in range(2):
            qg = qrows[t * 512:(t + 1) * 512]
            for kb in range(8):
                kr = krows[(8 * t + kb) * P:(8 * t + kb + 1) * P]
                mask[t, kb] = np.where(kr[:, None] <= qg[None, :], 0.0, NEG)
        in_maps.append({
            "tgt_t": tgt_t,
            "tgto": tgto,
            "src_t": src_t,
            "mask": mask.astype(BF),
            **w_common,
        })
    return in_maps


def assemble_output(results):
    out = np.empty((B, S, E), np.float32)
    for c in range(NCORES):
        b, h = c // 2, c % 2
        arr = np.asarray(results[c]["out_t"])  # [P, NE*RO]
        a = arr.reshape(P, NE, 8, P).transpose(2, 3, 1, 0).reshape(8, P, E)
        qrows = _own_rows(h)
        for j in range(8):
            out[b, qrows[j * P:(j + 1) * P], :] = a[j]
    return out


def get_nc():
    if "nc" not in _NC_CACHE:
        _NC_CACHE["nc"] = _build_program()
    return _NC_CACHE["nc"]


def _axon_reset():
    """Recover a wedged remote NeuronCore (NRT_EXEC_UNIT_UNRECOVERABLE)."""
    try:
        import ctypes
        lib = ctypes.CDLL("/opt/axon/libaxon_pjrt.so")
        lib.axon_reset.restype = ctypes.c_int64
        lib.axon_reset()
    except Exception:
        pass


def kernel(**inputs):
    global LAST_RESULTS
    in_maps = make_in_maps(inputs)
    nc = get_nc()
    last_err = None
    for attempt in range(3):
        try:
            res = run_bass_kernel_spmd(nc, in_maps, list(range(NCORES)))
            break
        except Exception as e:  # wedged device -> reset + retry
            last_err = e
            _axon_reset()
    else:
        raise last_err
    LAST_RESULTS = res
    return assemble_output(res.results)
